# revision 1
# baseline (speedup 1.0000x reference)
"""Capsule-routing kernel for Trainium2 (8 NeuronCores, Bass/Tile).

Problem: nn_ClfCapsule — B=256, INPUT_ATOMS=8, MUL=1024, K=20, O=16, 3 routing
iterations.

u_hat[b,m,k,o] = sum_a W[k,o,a]*xt[b,m,a] (335 MB) is never materialized:
  t[b,k,a]  = sum_m c[m,k] * xt[b,m,a]          (TensorE, contract m=1024)
  s[b,k,o]  = sum_a W[k,o,a] * t[b,k,a]         (DVE mul+segment-reduce)
  v         = squash_over_k(s)
  g[b,k,a]  = sum_o W[k,o,a] * v[b,k,o]         (DVE mul+segment-reduce)
  bU[m,k]   = sum_{b,a} xt[b,m,a] * g[b,k,a]    (TensorE, contract (b,a))
Data-parallel over batch (32/core); bU partials AllReduced after iters 1,2.

Perf notes:
- softmax needs no max-subtraction (|b_ij| < 1) and exp is a DVE Horner
  polynomial — avoids ACT exp<->sqrt table swaps (1.28us each; Exp and Sqrt
  are never in the same act-func set).
- glue runs in a (o_hi=4, b=32)-on-partitions layout: 128 DVE lanes busy
  instead of 32.  W is host-replicated per (o_hi, b) partition.
- iteration 1's softmax is uniform (b=0), so t1 = rowsum(x)/1024 via a
  ones-matmul, skipping the whole softmax.
"""

import numpy as np

B, A, M = 256, 8, 1024
K, O = 20, 16
NCORES = 8
BLOC = B // NCORES  # 32
MC = M // 128       # 8 m-chunks
OH, OL = 4, 4       # o = oh*4 + ol; oh lives on partition groups

_prog_cache = {}
USE_COLLECTIVES = True  # debug switch: False replaces AllReduce with local copy

# r := exp(x)-1 ~= x + x^2/2 + ... + x^5/120 via the recurrence
# r0 = x/120; r_{k+1} = (r_k + c)*x  with c in _EXPC (exact to degree 5;
# |x| < 0.9 here -> abs error < 8e-4, rel error on softmax ~3e-4,
# far under the accuracy gate).
_EXPC = [1.0 / 24, 1.0 / 6, 1.0 / 2, 1.0]


def _build_program(n_reps=1):
    """n_reps > 1 replicates the computation inside one NEFF for
    wall-clock benchmarking (chained bass_exec calls get CSE'd by XLA)."""
    import concourse.bacc as bacc
    import concourse.mybir as mybir
    import concourse.tile as tile

    dt = mybir.dt.float32
    nc = bacc.Bacc("TRN2", target_bir_lowering=False, debug=False,
                   num_devices=NCORES)

    # Host-prepped per-core DRAM inputs (see _host_prep):
    #   xm[p, mc, q] = xt[b, mc*128+p, a]  with q = a*32 + b   (m on partitions)
    #   xf[p, c, m]  = xt[b, m, a]         with a = c*4 + p//32, b = p%32
    #   ws[(a2,b), k, o, c]  = W[k, o, c*4+a2]
    #   wg[(a2,b), c, k, o]  = W[k, o, c*4+a2] / 256
    xm_d = nc.dram_tensor("xm", [128, MC, 2 * 128], dt, kind="ExternalInput")
    xf_d = nc.dram_tensor("xf", [128, 2, M], dt, kind="ExternalInput")
    ws_d = nc.dram_tensor("ws", [128, K, O, 2], dt, kind="ExternalInput")
    wg_d = nc.dram_tensor("wg", [128, 2, K, O], dt, kind="ExternalInput")
    # eb[a2*32+b, b'] = (b == b'): sums the 4 a2 partition groups via PE
    eb_d = nc.dram_tensor("eb", [128, BLOC], dt, kind="ExternalInput")
    out_d = nc.dram_tensor("out", [BLOC, K, O], dt, kind="ExternalOutput")

    SQRT = mybir.ActivationFunctionType.Sqrt
    ADD = mybir.AluOpType.add
    MULT = mybir.AluOpType.mult
    AXX = mybir.AxisListType.X
    RG = [list(range(NCORES))]

    with tile.TileContext(nc) as tc:
        with (
            tc.tile_pool(name="const", bufs=1) as cpool,
            tc.tile_pool(name="work", bufs=2) as wpool,
            tc.tile_pool(name="ps_tt", bufs=2, space="PSUM") as ps_tt,
            tc.tile_pool(name="ps_small", bufs=1, space="PSUM") as ps_small,
            tc.tile_pool(name="ps_bu", bufs=2, space="PSUM") as ps_bu,
            tc.tile_pool(name="dram", bufs=2, space="DRAM") as dpool,
        ):
            ones128 = cpool.tile([128, 1], dt)
            ones1 = cpool.tile([1, 128], dt)
            eb_sb = cpool.tile([128, BLOC], dt)
            nc.vector.memset(ones128[:], 1.0)
            nc.vector.memset(ones1[:], 1.0)
            nc.sync.dma_start(eb_sb[:], eb_d[:])

            for _rep in range(n_reps):
              xm_sb = wpool.tile([128, MC, 256], dt, name="xm_sb")
              xf_sb = wpool.tile([128, 2, M], dt, name="xf_sb")
              ws_sb = wpool.tile([128, K, O, 2], dt, name="ws_sb")
              wg_sb = wpool.tile([128, 2, K, O], dt, name="wg_sb")
              b_sb = wpool.tile([128, MC, K], dt, name="b_sb")

              # xm first and in halves (iter-1's ones-matmuls start on the
              # first half); xf last (only needed at matmul2); ws/wg on the
              # gpsimd queue so they overlap the xm transfer.
              nc.sync.dma_start(xm_sb[:, 0:MC // 2], xm_d[:, 0:MC // 2])
              nc.sync.dma_start(xm_sb[:, MC // 2:], xm_d[:, MC // 2:])
              nc.gpsimd.dma_start(ws_sb[:], ws_d[:])
              nc.gpsimd.dma_start(wg_sb[:], wg_d[:])
              nc.sync.dma_start(xf_sb[:], xf_d[:])

              for it in range(3):
                first = (it == 0)
                kd = 1 if first else K  # t is k-independent in iter 1

                if first:
                    # c uniform = 1/1024: t1[q] = sum_m Xm[m, q] / 1024
                    tt_ps = []
                    for c in range(2):
                        tt = ps_tt.tile([128, K], dt, name=f"t0_{c}", tag="tt")
                        for mc in range(MC):
                            nc.tensor.matmul(
                                tt[:, :1],
                                xm_sb[:, mc, c * 128:(c + 1) * 128],
                                ones128[:],
                                start=(mc == 0), stop=(mc == MC - 1))
                        tt_ps.append(tt)
                else:
                    # ---- softmax over m: poly-exp on DVE (no ACT table) ----
                    r_sb = wpool.tile([128, MC, K], dt, name="r_sb")
                    nc.vector.tensor_scalar_mul(r_sb[:], b_sb[:], 1.0 / 120)
                    for ck in _EXPC[:-1]:
                        nc.vector.scalar_tensor_tensor(
                            r_sb[:], r_sb[:], float(ck), b_sb[:],
                            op0=ADD, op1=MULT)
                    # e = exp(b) = (r + 1) * b + ... final Horner step + 1
                    e_sb = wpool.tile([128, MC, K], dt, name="e_sb")
                    nc.vector.scalar_tensor_tensor(
                        e_sb[:], r_sb[:], 1.0, b_sb[:], op0=ADD, op1=MULT)
                    nc.vector.tensor_scalar_add(e_sb[:], e_sb[:], 1.0)
                    # denom[k] = sum_m e  (ones-matmul + cross-chunk reduce)
                    dn_ps = ps_small.tile([1, MC, K], dt, name="dn_ps", tag="dn")
                    nc.tensor.matmul(dn_ps[:], ones128[:], e_sb[:])
                    dsum = wpool.tile([1, K], dt, name="dsum")
                    nc.vector.tensor_reduce(
                        dsum[:], dn_ps[:].transpose([0, 2, 1]), axis=AXX, op=ADD)
                    rcp = wpool.tile([1, K], dt, name="rcp")
                    nc.vector.reciprocal(rcp[:], dsum[:])
                    rb_ps = ps_small.tile([128, K], dt, name="rb_ps", tag="rb")
                    nc.tensor.matmul(rb_ps[:], ones1[:], rcp[:])
                    rb_sb = wpool.tile([128, K], dt, name="rb_sb")
                    nc.vector.tensor_copy(rb_sb[:], rb_ps[:])

                    # ---- matmul1 on unnormalized e (1/denom folded in below)
                    tt_ps = []
                    for c in range(2):
                        tt = ps_tt.tile([128, K], dt, name=f"tt{c}", tag="tt")
                        for mc in range(MC):
                            nc.tensor.matmul(
                                tt[:],
                                xm_sb[:, mc, c * 128:(c + 1) * 128],
                                e_sb[:, mc, :],
                                start=(mc == 0), stop=(mc == MC - 1))
                        tt_ps.append(tt)

                # ---- t4c[(a2,b), k, c] <- tt_ps[c] * (1/denom), one op/chunk --
                t4c = wpool.tile([128, kd, 2], dt, name="t4c", tag="t4c")
                for c in range(2):
                    if first:
                        nc.vector.tensor_scalar_mul(
                            t4c[:, :, c], tt_ps[c][:, :1], 1.0 / M)
                    else:
                        nc.vector.tensor_tensor(
                            t4c[:, :, c], tt_ps[c][:, :], rb_sb[:, :], op=MULT)

                # ---- s[b, k, o] = sum_{c, a2} ws * t  (c in-op, a2 via PE) --
                sP = wpool.tile([128, K, O, 2], dt, name="sP")
                nc.vector.tensor_tensor(
                    sP[:], t4c[:].unsqueeze(2).broadcast_to([128, K, O, 2]),
                    ws_sb[:], op=MULT)
                sp2 = wpool.tile([128, K, O], dt, name="sp2")
                nc.vector.tensor_reduce(sp2[:], sP[:], axis=AXX, op=ADD)
                s_b = ps_small.tile([BLOC, K, O], dt, name="s_b", tag="s_b")
                nc.tensor.matmul(s_b[:], eb_sb[:], sp2[:])

                # ---- squash over k ----
                sq = wpool.tile([BLOC, K, O], dt, name="sq")
                nc.scalar.square(sq[:], s_b[:])
                ms = wpool.tile([BLOC, O], dt, name="ms")
                nc.vector.tensor_reduce(ms[:], sq[:].transpose([0, 2, 1]),
                                        axis=AXX, op=ADD)
                mag = wpool.tile([BLOC, O], dt, name="mag")
                nc.scalar.sqrt(mag[:], ms[:])
                den = wpool.tile([BLOC, O], dt, name="den")
                nc.vector.tensor_scalar_add(den[:], ms[:], 1.0)
                rd = wpool.tile([BLOC, O], dt, name="rd")
                nc.vector.reciprocal(rd[:], den[:])
                f_b = wpool.tile([BLOC, O], dt, name="f_b")
                nc.vector.tensor_tensor(f_b[:], mag[:], rd[:], op=MULT)

                if it == 2:
                    vout = wpool.tile([BLOC, K, O], dt, name="vout")
                    nc.vector.tensor_tensor(
                        vout[:], s_b[:],
                        f_b[:].unsqueeze(1).broadcast_to([BLOC, K, O]),
                        op=MULT)
                    nc.sync.dma_start(out_d[:], vout[:])
                    continue

                # ---- v replicated over a2 partition groups ----
                v_rep = wpool.tile([128, K, O], dt, name="v_rep")
                nc.vector.tensor_tensor(
                    v_rep[0:32], s_b[:],
                    f_b[:].unsqueeze(1).broadcast_to([BLOC, K, O]), op=MULT)
                nc.vector.tensor_copy(v_rep[32:64], v_rep[0:32])
                nc.vector.tensor_copy(v_rep[64:128], v_rep[0:64])

                # ---- Gp[(a2,b), c, k] = sum_o wg[(a2,b), c, k, o] * v[b,k,o]
                gP = wpool.tile([128, 2, K, O], dt, name="gP")
                nc.vector.tensor_tensor(
                    gP[:], v_rep[:].unsqueeze(1).broadcast_to([128, 2, K, O]),
                    wg_sb[:], op=MULT)
                Gp = wpool.tile([128, 2, K], dt, name="Gp")
                nc.vector.tensor_reduce(Gp[:], gP[:], axis=AXX, op=ADD)

                # ---- matmul2: bU[m, k] = sum_q Xf[q, m] * Gp[q, k] ----
                bu = ps_bu.tile([128, MC, K], dt, name="bu", tag="bu")
                for mt in range(MC):
                    for c in range(2):
                        nc.tensor.matmul(
                            bu[:, mt, :],
                            xf_sb[:, c, mt * 128:(mt + 1) * 128],
                            Gp[:, c, :],
                            start=(c == 0), stop=(c == 1))

                bnew = wpool.tile([128, MC, K], dt, name="bnew")
                nc.vector.tensor_copy(bnew[:], bu[:])

                # ---- AllReduce b_ij update over the 8 cores ----
                cc_in = dpool.tile([128, MC, K], dt, name="cc_in")
                cc_out = dpool.tile([128, MC, K], dt, name="cc_out",
                                    addr_space="Shared")
                nc.sync.dma_start(cc_in[:], bnew[:])
                if USE_COLLECTIVES:
                    nc.gpsimd.collective_compute(
                        "AllReduce", ADD, replica_groups=RG,
                        ins=[cc_in[:].opt()], outs=[cc_out[:].opt()])
                else:
                    nc.sync.dma_start(cc_out[:], cc_in[:])
                ar_sb = wpool.tile([128, MC, K], dt, name="ar_sb")
                nc.sync.dma_start(ar_sb[:], cc_out[:])
                if first:
                    nc.vector.tensor_copy(b_sb[:], ar_sb[:])
                else:
                    nc.vector.tensor_tensor(b_sb[:], b_sb[:], ar_sb[:], op=ADD)

    nc.compile()
    return nc


def _host_prep(x):
    """Build the 8 per-core input maps from the full x [B, A, M]."""
    x = np.ascontiguousarray(x, dtype=np.float32)
    xt = x.reshape(B, M, A)  # faithful to reference's reshape (NOT a transpose)
    in_maps = []
    for i in range(NCORES):
        xi = xt[i * BLOC:(i + 1) * BLOC]              # [32, 1024, 8]
        # xm[p, mc, a*32+b]
        xm = xi.transpose(1, 2, 0).reshape(MC, 128, A, BLOC)
        xm = np.ascontiguousarray(xm.transpose(1, 0, 2, 3)).reshape(128, MC, 256)
        # xf[a'*32+b, c, m] with a = c*4+a'
        xf = xi.transpose(2, 0, 1).reshape(2, 4, BLOC, M)
        xf = np.ascontiguousarray(xf.transpose(1, 2, 0, 3)).reshape(128, 2, M)
        in_maps.append({"xm": xm, "xf": xf})
    return in_maps


def _host_w(W):
    """ws[(a2,b), k, o, c] = W[k, o, c*4+a2];
    wg[(a2,b), c, k, o] = W[k, o, c*4+a2] / B."""
    W = np.ascontiguousarray(W, dtype=np.float32)
    wss = W.reshape(K, O, 2, 4).transpose(3, 0, 1, 2)    # [a2, k, o, c]
    ws = np.ascontiguousarray(
        np.broadcast_to(wss[:, None], (4, BLOC, K, O, 2))).reshape(
            128, K, O, 2)
    wgs = (W / B).transpose(2, 0, 1).reshape(2, 4, K, O)  # [c, a2, k, o]
    wg = np.ascontiguousarray(
        np.broadcast_to(wgs.transpose(1, 0, 2, 3)[:, None],
                        (4, BLOC, 2, K, O))).reshape(128, 2, K, O)
    eb = np.tile(np.eye(BLOC, dtype=np.float32), (4, 1))
    return {"ws": ws, "wg": wg, "eb": eb}


def _run(x, W, trace=False):
    from concourse import bass_utils

    if "nc" not in _prog_cache:
        _prog_cache["nc"] = _build_program()
    nc = _prog_cache["nc"]

    consts = _host_w(W)
    in_maps = _host_prep(x)
    for m in in_maps:
        m.update(consts)

    res = bass_utils.run_bass_kernel_spmd(
        nc, in_maps, core_ids=list(range(NCORES)), trace=trace)
    out = np.concatenate([r["out"] for r in res.results], axis=0)
    return out.reshape(B, K, O, 1).astype(np.float32), res


def kernel(x, W):
    out, _ = _run(x, W)
    return out



# revision 14
# speedup vs baseline: 1.2844x; 1.2844x over previous
"""Capsule-routing kernel for Trainium2 (8 NeuronCores, Bass/Tile).

Problem: nn_ClfCapsule — B=256, INPUT_ATOMS=8, MUL=1024, K=20, O=16, 3 routing
iterations.

u_hat[b,m,k,o] = sum_a W[k,o,a]*xt[b,m,a] (335 MB) is never materialized:
  t[b,k,a]  = sum_m c[m,k] * xt[b,m,a]          (TensorE, contract m=1024)
  s[b,k,o]  = sum_a W[k,o,a] * t[b,k,a]         (DVE mul + add, a2-sum on PE)
  v         = squash_over_k(s)
  g[b,k,a]  = sum_o W[k,o,a] * v[b,k,o]         (DVE mul + reduce)
  bU[m,k]   = sum_{b,a} xt[b,m,a] * g[b,k,a]    (TensorE, contract (b,a))
Data-parallel over batch (32/core); bU partials AllReduced after iters 1,2.

Perf design (vs the fp32 baseline, 89.2us -> target ~60us):
- fp16 datapath: matmuls run 1 cycle/row instead of 4 (fp32), and DVE
  elementwise ops get the 2x/4x 16-bit modes.  fp16 (10-bit mantissa) keeps
  the pipeline rel-err ~1e-3, far under the 2e-2 gate (verified in numpy).
- softmax exp via a tuned degree-3 recurrence (4 DVE ops; |b| <= 0.9).
- softmax denominator via 8 accumulating PE matmuls (no DVE cross-chunk
  reduce); 1/denom broadcast by a ones-matmul.
- squash runs on all 128 partitions: the a2-group sum uses a [128,128]
  block-identity matmul that REPLICATES s into all 4 partition groups,
  removing the v-replication copies and the 32-partition squash.
- ACT engine is used only for sqrt; a dummy sqrt at t=0 preloads the
  activation table off the critical path.
- b_ij never lives on its own: the AllReduce input is bu + b_prev/8, so the
  post-collective path is DMA -> poly directly.
"""

import numpy as np

B, A, M = 256, 8, 1024
K, O = 20, 16
NCORES = 8
BLOC = B // NCORES  # 32
MC = M // 128       # 8 m-chunks

_prog_cache = {}
USE_COLLECTIVES = True  # debug switch: False replaces AllReduce with local copy

# e^x ~= ((x*C2 + C1)*x + C0)*x + 1, least-squares fit of (e^x-1)/x on
# [-0.9, 0.9] (|b_ij| <= 0.84 empirically); poly rel err < 2.6e-2 at the
# edges, pipeline rel err ~1e-3 (fp16 sim vs fp64 oracle).
C2, C1, C0 = 0.17254924561157742, 0.5206554848971009, 0.9995207222919545


def _build_program(n_reps=1):
    import concourse.bacc as bacc
    import concourse.mybir as mybir
    import concourse.tile as tile

    f32 = mybir.dt.float32
    f16 = mybir.dt.float16
    nc = bacc.Bacc("TRN2", target_bir_lowering=False, debug=False,
                   num_devices=NCORES)

    # Host-prepped per-core DRAM inputs (see _host_prep / _host_w):
    #   xm[p, mc, q]      = xt[b, mc*128+p, a]   with q = a*32 + b
    #   xf[a2*32+b, c, m] = xt[b, m, c*4+a2]
    #   ws[(a2,b), c, o, k] = W[k, o, c*4+a2]
    #   wg[(a2,b), c, k, o] = W[k, o, c*4+a2] / 256
    #   eb4[(a2,b), (a2',b')] = (b == b')  (sums a2 groups AND replicates)
    xm_d = nc.dram_tensor("xm", [128, MC, 2 * 128], f16, kind="ExternalInput")
    xf_d = nc.dram_tensor("xf", [128, 2, M], f16, kind="ExternalInput")
    ws_d = nc.dram_tensor("ws", [128, 2, O, K], f16, kind="ExternalInput")
    wg_d = nc.dram_tensor("wg", [128, 2, K, O], f16, kind="ExternalInput")
    eb_d = nc.dram_tensor("eb4", [128, 128], f16, kind="ExternalInput")
    out_d = nc.dram_tensor("out", [BLOC, K, O], f16, kind="ExternalOutput")

    ADD = mybir.AluOpType.add
    MULT = mybir.AluOpType.mult
    AXX = mybir.AxisListType.X
    RG = [list(range(NCORES))]

    with tile.TileContext(nc) as tc:
        with (
            nc.allow_low_precision(
                reason="fp16 datapath validated vs fp64 oracle (rel ~1e-3)"),
            tc.tile_pool(name="const", bufs=1) as cpool,
            tc.tile_pool(name="work", bufs=2) as wpool,
            tc.tile_pool(name="ps_tt", bufs=2, space="PSUM") as ps_tt,
            tc.tile_pool(name="ps_small", bufs=1, space="PSUM") as ps_small,
            tc.tile_pool(name="ps_s4", bufs=2, space="PSUM") as ps_s4,
            tc.tile_pool(name="ps_bu", bufs=2, space="PSUM") as ps_bu,
            tc.tile_pool(name="dram", bufs=2, space="DRAM") as dpool,
        ):
            ones128 = cpool.tile([128, 1], f16)
            ones1 = cpool.tile([1, 128], f16)
            eb_sb = cpool.tile([128, 128], f16)
            sqd = cpool.tile([1, 1], f32)
            nc.vector.memset(ones128[:], 1.0)
            nc.vector.memset(ones1[:], 1.0)
            nc.vector.memset(sqd[:], 1.0)
            # dummy sqrt: pulls the ACT Sqrt table load off the critical path
            nc.scalar.sqrt(sqd[:], sqd[:])

            for _rep in range(n_reps):
              xm_sb = wpool.tile([128, MC, 256], f16, name="xm_sb")
              xf_sb = wpool.tile([128, 2, M], f16, name="xf_sb")
              ws_sb = wpool.tile([128, 2, O, K], f16, name="ws_sb")
              wg_sb = wpool.tile([128, 2, K, O], f16, name="wg_sb")

              # xm halves first, then xf, all on the SP queue: the DMA engine
              # track is a FIFO, so issue order keeps xm (needed first) ahead
              # of xf (needed only at matmul2).  ws/wg/eb4 ride the gpsimd
              # SWDGE path, which bypasses the HWDGE queue.
              nc.sync.dma_start(xm_sb[:, 0:MC // 2], xm_d[:, 0:MC // 2])
              nc.sync.dma_start(xm_sb[:, MC // 2:], xm_d[:, MC // 2:])
              nc.sync.dma_start(xf_sb[:], xf_d[:])
              nc.gpsimd.dma_start(ws_sb[:], ws_d[:])
              nc.gpsimd.dma_start(wg_sb[:], wg_d[:])
              nc.gpsimd.dma_start(eb_sb[:], eb_d[:])

              ar_sb = None
              for it in range(3):
                first = (it == 0)

                if first:
                    # c uniform = 1/M: t1[q] = sum_m xm[m, q] / M, k-indep.
                    ttl = []
                    for c in range(2):
                        t_ = ps_tt.tile([128, 1], f32, name=f"ttl{c}", tag="tt")
                        for mc in range(MC):
                            nc.tensor.matmul(
                                t_[:], xm_sb[:, mc, c * 128:(c + 1) * 128],
                                ones128[:],
                                start=(mc == 0), stop=(mc == MC - 1))
                        ttl.append(t_)
                    t1sc = wpool.tile([128, 2], f32, name="t1sc")
                    for c in range(2):
                        nc.vector.tensor_scalar_mul(
                            t1sc[:, c:c + 1], ttl[c][:], 1.0 / M)
                    # sP via per-partition-scalar mult (TSP 4x mode)
                    sP = wpool.tile([128, 2, O, K], f16, name="sP")
                    nc.vector.tensor_scalar(
                        sP[:, 0], ws_sb[:, 0], t1sc[:, 0:1], None, op0=MULT)
                    nc.vector.tensor_scalar(
                        sP[:, 1], ws_sb[:, 1], t1sc[:, 1:2], None, op0=MULT)
                else:
                    # ---- softmax numerator: e = poly(b), b = ar_sb (fp32).
                    # e = (C0*b + 1) + b^2*(C2*b + C1) with only TSP/TT ops
                    # (scalar_tensor_tensor never gets the 16-bit fast modes).
                    b16 = wpool.tile([128, MC, K], f16, name="b16")
                    nc.vector.tensor_scalar_mul(b16[:], ar_sb[:], 1.0)
                    b2 = wpool.tile([128, MC, K], f16, name="b2")
                    nc.vector.tensor_tensor(b2[:], b16[:], b16[:], op=MULT)
                    w_sb = wpool.tile([128, MC, K], f16, name="w_sb")
                    nc.vector.tensor_scalar(
                        w_sb[:], b16[:], C2, C1, op0=MULT, op1=ADD)
                    u_sb = wpool.tile([128, MC, K], f16, name="u_sb")
                    nc.vector.tensor_scalar(
                        u_sb[:], b16[:], C0, 1.0, op0=MULT, op1=ADD)
                    p_sb = wpool.tile([128, MC, K], f16, name="p_sb")
                    nc.vector.tensor_tensor(p_sb[:], b2[:], w_sb[:], op=MULT)
                    e_sb = wpool.tile([128, MC, K], f16, name="e_sb")
                    nc.vector.tensor_tensor(e_sb[:], u_sb[:], p_sb[:], op=ADD)

                    # ---- denom: dn[k] = sum_m e (8 accumulating matmuls)
                    dn = ps_small.tile([1, K], f32, name="dn", tag="dn")
                    for mc in range(MC):
                        nc.tensor.matmul(dn[:], ones128[:], e_sb[:, mc, :],
                                         start=(mc == 0), stop=(mc == MC - 1))
                    rcp16 = wpool.tile([1, K], f16, name="rcp16")
                    nc.vector.reciprocal(rcp16[:], dn[:])

                    # ---- matmul1 on unnormalized e
                    tt = []
                    for c in range(2):
                        t_ = ps_tt.tile([128, K], f32, name=f"tt{c}", tag="tt")
                        for mc in range(MC):
                            nc.tensor.matmul(
                                t_[:], xm_sb[:, mc, c * 128:(c + 1) * 128],
                                e_sb[:, mc, :],
                                start=(mc == 0), stop=(mc == MC - 1))
                        tt.append(t_)

                    # 1/denom broadcast to 128 partitions via ones-matmul
                    rb = ps_small.tile([128, K], f32, name="rb", tag="rb")
                    nc.tensor.matmul(rb[:], ones1[:], rcp16[:])
                    rb16 = wpool.tile([128, K], f16, name="rb16")
                    nc.vector.tensor_copy(rb16[:], rb[:])

                    t4c = wpool.tile([128, 2, K], f16, name="t4c")
                    for c in range(2):
                        nc.vector.tensor_tensor(
                            t4c[:, c], tt[c][:], rb16[:], op=MULT)

                    # ---- s partials: sP = t4c (bcast over o) * ws ----
                    sP = wpool.tile([128, 2, O, K], f16, name="sP")
                    nc.vector.tensor_tensor(
                        sP[:],
                        t4c[:].unsqueeze(2).broadcast_to([128, 2, O, K]),
                        ws_sb[:], op=MULT)

                # ---- c-sum + a2-group sum + replicate on PE:
                #      s4[(a2',b), k, o] = sum_c sum_a2 sP ----
                s4 = ps_s4.tile([128, K, O], f32, name="s4", tag="s4")
                for c in range(2):
                    nc.tensor.matmul(
                        s4[:], eb_sb[:], sP[:, c].transpose([0, 2, 1]),
                        start=(c == 0), stop=(c == 1))

                # ---- squash over k (on all 128 partitions) ----
                sq = wpool.tile([128, K, O], f16, name="sq")
                nc.scalar.square(sq[:], s4[:])
                ms = wpool.tile([128, O], f16, name="ms")
                nc.vector.tensor_reduce(ms[:], sq[:].transpose([0, 2, 1]),
                                        axis=AXX, op=ADD)
                mag = wpool.tile([128, O], f16, name="mag")
                nc.scalar.sqrt(mag[:], ms[:])
                den = wpool.tile([128, O], f16, name="den")
                nc.vector.tensor_scalar_add(den[:], ms[:], 1.0)
                rd = wpool.tile([128, O], f16, name="rd")
                nc.vector.reciprocal(rd[:], den[:])
                fb = wpool.tile([128, O], f16, name="fb")
                nc.vector.tensor_tensor(fb[:], mag[:], rd[:], op=MULT)

                if it == 2:
                    v4 = wpool.tile([BLOC, K, O], f16, name="vout")
                    nc.vector.tensor_tensor(
                        v4[:], s4[0:BLOC],
                        fb[0:BLOC].unsqueeze(1).broadcast_to([BLOC, K, O]),
                        op=MULT)
                    nc.sync.dma_start(out_d[:], v4[:])
                    continue

                v4 = wpool.tile([128, K, O], f16, name="v4")
                nc.vector.tensor_tensor(
                    v4[:], s4[:],
                    fb[:].unsqueeze(1).broadcast_to([128, K, O]), op=MULT)

                # ---- Gp[(a2,b), c, k] = sum_o wg * v.  The o-reduce is
                # split per c so matmul2's c=0 chain can start while the
                # c=1 half still reduces. ----
                gP = wpool.tile([128, 2, K, O], f16, name="gP")
                nc.vector.tensor_tensor(
                    gP[:], v4[:].unsqueeze(1).broadcast_to([128, 2, K, O]),
                    wg_sb[:], op=MULT)
                Gp = wpool.tile([128, 2, K], f16, name="Gp")
                nc.vector.tensor_reduce(Gp[:, 0], gP[:, 0], axis=AXX, op=ADD)
                nc.vector.tensor_reduce(Gp[:, 1], gP[:, 1], axis=AXX, op=ADD)

                # ---- matmul2: bu[m, k] = sum_q xf[q, m] * Gp[q, k] ----
                # (mt-major: one PSUM accumulation group open at a time)
                bu = ps_bu.tile([128, MC, K], f32, name="bu", tag="bu")
                for mt in range(MC):
                    for c in range(2):
                        nc.tensor.matmul(
                            bu[:, mt, :],
                            xf_sb[:, c, mt * 128:(mt + 1) * 128],
                            Gp[:, c, :],
                            start=(c == 0), stop=(c == 1))

                # ---- AllReduce of (bu + b_prev/8) over the 8 cores ----
                cc_in = dpool.tile([128, MC, K], f32, name="cc_in")
                cc_out = dpool.tile([128, MC, K], f32, name="cc_out",
                                    addr_space="Shared")
                ccs = wpool.tile([128, MC, K], f32, name="ccs")
                if first:
                    nc.vector.tensor_copy(ccs[:], bu[:])
                else:
                    nc.vector.scalar_tensor_tensor(
                        ccs[:], ar_sb[:], 1.0 / NCORES, bu[:],
                        op0=MULT, op1=ADD)
                ar_sb = wpool.tile([128, MC, K], f32, name="ar_sb")
                nc.sync.dma_start(cc_in[:], ccs[:])
                if USE_COLLECTIVES:
                    nc.gpsimd.collective_compute(
                        "AllReduce", ADD, replica_groups=RG,
                        ins=[cc_in[:].opt()], outs=[cc_out[:].opt()])
                    nc.sync.dma_start(ar_sb[:], cc_out[:])
                else:
                    nc.sync.dma_start(ar_sb[:], cc_in[:])

    nc.compile()
    return nc


def _host_prep(x):
    """Build the 8 per-core input maps from the full x [B, A, M]."""
    x = np.ascontiguousarray(x, dtype=np.float32)
    xt = x.reshape(B, M, A)  # faithful to reference's reshape (NOT a transpose)
    in_maps = []
    for i in range(NCORES):
        xi = xt[i * BLOC:(i + 1) * BLOC]              # [32, 1024, 8]
        # xm[p, mc, a*32+b]
        xm = xi.transpose(1, 2, 0).reshape(MC, 128, A, BLOC)
        xm = np.ascontiguousarray(
            xm.transpose(1, 0, 2, 3)).reshape(128, MC, 256).astype(np.float16)
        # xf[a2*32+b, c, m] with a = c*4+a2
        xf = xi.transpose(2, 0, 1).reshape(2, 4, BLOC, M)
        xf = np.ascontiguousarray(
            xf.transpose(1, 2, 0, 3)).reshape(128, 2, M).astype(np.float16)
        in_maps.append({"xm": xm, "xf": xf})
    return in_maps


def _host_w(W):
    """ws[(a2,b), c, o, k] = W[k, o, c*4+a2];
    wg[(a2,b), c, k, o] = W[k, o, c*4+a2] / B;
    eb4[(a2,b), (a2',b')] = (b == b')."""
    W = np.ascontiguousarray(W, dtype=np.float32)
    wss = W.reshape(K, O, 2, 4).transpose(3, 2, 1, 0)     # [a2, c, o, k]
    ws = np.ascontiguousarray(
        np.broadcast_to(wss[:, None], (4, BLOC, 2, O, K))).reshape(
            128, 2, O, K).astype(np.float16)
    wgs = (W / B).reshape(K, O, 2, 4).transpose(3, 2, 0, 1)  # [a2, c, k, o]
    wg = np.ascontiguousarray(
        np.broadcast_to(wgs[:, None], (4, BLOC, 2, K, O))).reshape(
            128, 2, K, O).astype(np.float16)
    eb4 = np.tile(np.eye(BLOC, dtype=np.float16), (4, 4))
    return {"ws": ws, "wg": wg, "eb4": eb4}


def _run(x, W, trace=False):
    from concourse import bass_utils

    if "nc" not in _prog_cache:
        _prog_cache["nc"] = _build_program()
    nc = _prog_cache["nc"]

    consts = _host_w(W)
    in_maps = _host_prep(x)
    for m in in_maps:
        m.update(consts)

    res = bass_utils.run_bass_kernel_spmd(
        nc, in_maps, core_ids=list(range(NCORES)), trace=trace)
    out = np.concatenate(
        [np.asarray(r["out"], np.float32) for r in res.results], axis=0)
    return out.reshape(B, K, O, 1), res


def kernel(x, W):
    out, _ = _run(x, W)
    return out


# revision 21
# speedup vs baseline: 1.2964x; 1.0093x over previous
"""Capsule-routing kernel for Trainium2 (8 NeuronCores, Bass/Tile).

Problem: nn_ClfCapsule — B=256, INPUT_ATOMS=8, MUL=1024, K=20, O=16, 3 routing
iterations.

u_hat[b,m,k,o] = sum_a W[k,o,a]*xt[b,m,a] (335 MB) is never materialized:
  t[b,k,a]  = sum_m c[m,k] * xt[b,m,a]          (TensorE, contract m=1024)
  s[b,k,o]  = sum_a W[k,o,a] * t[b,k,a]         (DVE mul + add, a2-sum on PE)
  v         = squash_over_k(s)
  g[b,k,a]  = sum_o W[k,o,a] * v[b,k,o]         (DVE mul + reduce)
  bU[m,k]   = sum_{b,a} xt[b,m,a] * g[b,k,a]    (TensorE, contract (b,a))
Data-parallel over batch (32/core); bU partials AllReduced after iters 1,2.

Perf design (vs the fp32 baseline, 89.2us -> target ~60us):
- fp16 datapath: matmuls run 1 cycle/row instead of 4 (fp32), and DVE
  elementwise ops get the 2x/4x 16-bit modes.  fp16 (10-bit mantissa) keeps
  the pipeline rel-err ~1e-3, far under the 2e-2 gate (verified in numpy).
- softmax exp via a tuned degree-3 recurrence (4 DVE ops; |b| <= 0.9).
- softmax denominator via 8 accumulating PE matmuls (no DVE cross-chunk
  reduce); 1/denom broadcast by a ones-matmul.
- squash runs on all 128 partitions: the a2-group sum uses a [128,128]
  block-identity matmul that REPLICATES s into all 4 partition groups,
  removing the v-replication copies and the 32-partition squash.
- ACT engine is used only for sqrt; a dummy sqrt at t=0 preloads the
  activation table off the critical path.
- b_ij never lives on its own: the AllReduce input is bu + b_prev/8, so the
  post-collective path is DMA -> poly directly.
"""

import numpy as np

B, A, M = 256, 8, 1024
K, O = 20, 16
NCORES = 8
BLOC = B // NCORES  # 32
MC = M // 128       # 8 m-chunks

_prog_cache = {}
USE_COLLECTIVES = True  # debug switch: False replaces AllReduce with local copy

# e^x ~= ((x*C2 + C1)*x + C0)*x + 1, least-squares fit of (e^x-1)/x on
# [-0.9, 0.9] (|b_ij| <= 0.84 empirically); poly rel err < 2.6e-2 at the
# edges, pipeline rel err ~1e-3 (fp16 sim vs fp64 oracle).
C2, C1, C0 = 0.17254924561157742, 0.5206554848971009, 0.9995207222919545


def _build_program(n_reps=1):
    import concourse.bacc as bacc
    import concourse.mybir as mybir
    import concourse.tile as tile

    f32 = mybir.dt.float32
    f16 = mybir.dt.float16
    nc = bacc.Bacc("TRN2", target_bir_lowering=False, debug=False,
                   num_devices=NCORES)

    # Host-prepped per-core DRAM inputs (see _host_prep / _host_w):
    #   xm[p, mc, q]      = xt[b, mc*128+p, a]   with q = a*32 + b
    #   xf[a2*32+b, c, m] = xt[b, m, c*4+a2]
    #   ws[(a2,b), c, o, k] = W[k, o, c*4+a2]
    #   wg[(a2,b), c, k, o] = W[k, o, c*4+a2] / 256
    #   eb4[(a2,b), (a2',b')] = (b == b')  (sums a2 groups AND replicates)
    xm_d = nc.dram_tensor("xm", [128, MC, 2 * 128], f16, kind="ExternalInput")
    xf_d = nc.dram_tensor("xf", [128, 2, M], f16, kind="ExternalInput")
    ws_d = nc.dram_tensor("ws", [128, 2, O, K], f16, kind="ExternalInput")
    wg_d = nc.dram_tensor("wg", [128, 2, K, O], f16, kind="ExternalInput")
    eb_d = nc.dram_tensor("eb4", [128, 128], f16, kind="ExternalInput")
    out_d = nc.dram_tensor("out", [BLOC, K, O], f16, kind="ExternalOutput")

    ADD = mybir.AluOpType.add
    MULT = mybir.AluOpType.mult
    AXX = mybir.AxisListType.X
    RG = [list(range(NCORES))]

    with tile.TileContext(nc) as tc:
        with (
            nc.allow_low_precision(
                reason="fp16 datapath validated vs fp64 oracle (rel ~1e-3)"),
            tc.tile_pool(name="const", bufs=1) as cpool,
            tc.tile_pool(name="work", bufs=2) as wpool,
            tc.tile_pool(name="ps_tt", bufs=2, space="PSUM") as ps_tt,
            tc.tile_pool(name="ps_small", bufs=1, space="PSUM") as ps_small,
            tc.tile_pool(name="ps_s4", bufs=2, space="PSUM") as ps_s4,
            tc.tile_pool(name="ps_bu", bufs=2, space="PSUM") as ps_bu,
            tc.tile_pool(name="dram", bufs=2, space="DRAM") as dpool,
        ):
            ones128 = cpool.tile([128, 1], f16)
            ones1 = cpool.tile([1, 128], f16)
            eb_sb = cpool.tile([128, 128], f16)
            sqd = cpool.tile([1, 1], f32)
            nc.vector.memset(ones128[:], 1.0)
            nc.vector.memset(ones1[:], 1.0)
            nc.vector.memset(sqd[:], 1.0)
            # dummy sqrt: pulls the ACT Sqrt table load off the critical path
            nc.scalar.sqrt(sqd[:], sqd[:])

            for _rep in range(n_reps):
              xm_sb = wpool.tile([128, MC, 256], f16, name="xm_sb")
              xf_sb = wpool.tile([128, 2, M], f16, name="xf_sb")
              ws_sb = wpool.tile([128, 2, O, K], f16, name="ws_sb")
              wg_sb = wpool.tile([128, 2, K, O], f16, name="wg_sb")

              # DMA-engine FIFO order target: xm, eb4, ws, xf, wg (by first
              # use).  xm+xf on the SP/HWDGE path; eb4/ws/wg on the gpsimd
              # SWDGE path whose descgen serializes ~1us apart, interleaving
              # them into the FIFO behind xm.
              nc.sync.dma_start(xm_sb[:], xm_d[:])
              nc.sync.dma_start(xf_sb[:], xf_d[:])
              nc.gpsimd.dma_start(eb_sb[:], eb_d[:])
              nc.gpsimd.dma_start(ws_sb[:], ws_d[:])
              nc.gpsimd.dma_start(wg_sb[:], wg_d[:])

              ar_sb = None
              for it in range(3):
                first = (it == 0)

                if first:
                    # c uniform = 1/M: t1[q] = sum_m xm[m, q] / M, k-indep.
                    ttl = []
                    for c in range(2):
                        t_ = ps_tt.tile([128, 1], f32, name=f"ttl{c}", tag="tt")
                        for mc in range(MC):
                            nc.tensor.matmul(
                                t_[:], xm_sb[:, mc, c * 128:(c + 1) * 128],
                                ones128[:],
                                start=(mc == 0), stop=(mc == MC - 1))
                        ttl.append(t_)
                    t1sc = wpool.tile([128, 2], f32, name="t1sc")
                    for c in range(2):
                        nc.vector.tensor_scalar_mul(
                            t1sc[:, c:c + 1], ttl[c][:], 1.0 / M)
                    # sP via per-partition-scalar mult (TSP 4x mode)
                    sP = wpool.tile([128, 2, O, K], f16, name="sP")
                    for c in range(2):
                        nc.vector.tensor_scalar(
                            sP[:, c], ws_sb[:, c], t1sc[:, c:c + 1],
                            None, op0=MULT)
                else:
                    # ---- softmax numerator: e = poly(b), b = ar_sb (fp32).
                    # e = (C0*b + 1) + b^2*(C2*b + C1) with only TSP/TT ops
                    # (scalar_tensor_tensor never gets the 16-bit fast modes).
                    b16 = wpool.tile([128, MC, K], f16, name="b16")
                    nc.vector.tensor_scalar_mul(b16[:], ar_sb[:], 1.0)
                    b2 = wpool.tile([128, MC, K], f16, name="b2")
                    nc.vector.tensor_tensor(b2[:], b16[:], b16[:], op=MULT)
                    w_sb = wpool.tile([128, MC, K], f16, name="w_sb")
                    nc.vector.tensor_scalar(
                        w_sb[:], b16[:], C2, C1, op0=MULT, op1=ADD)
                    u_sb = wpool.tile([128, MC, K], f16, name="u_sb")
                    nc.vector.tensor_scalar(
                        u_sb[:], b16[:], C0, 1.0, op0=MULT, op1=ADD)
                    p_sb = wpool.tile([128, MC, K], f16, name="p_sb")
                    nc.vector.tensor_tensor(p_sb[:], b2[:], w_sb[:], op=MULT)
                    e_sb = wpool.tile([128, MC, K], f16, name="e_sb")
                    nc.vector.tensor_tensor(e_sb[:], u_sb[:], p_sb[:], op=ADD)

                    # ---- denom: dn[k] = sum_m e (8 accumulating matmuls)
                    dn = ps_small.tile([1, K], f32, name="dn", tag="dn")
                    for mc in range(MC):
                        nc.tensor.matmul(dn[:], ones128[:], e_sb[:, mc, :],
                                         start=(mc == 0), stop=(mc == MC - 1))
                    rcp16 = wpool.tile([1, K], f16, name="rcp16")
                    nc.vector.reciprocal(rcp16[:], dn[:])

                    # ---- matmul1 on unnormalized e
                    tt = []
                    for c in range(2):
                        t_ = ps_tt.tile([128, K], f32, name=f"tt{c}", tag="tt")
                        for mc in range(MC):
                            nc.tensor.matmul(
                                t_[:], xm_sb[:, mc, c * 128:(c + 1) * 128],
                                e_sb[:, mc, :],
                                start=(mc == 0), stop=(mc == MC - 1))
                        tt.append(t_)

                    # 1/denom broadcast to 128 partitions via ones-matmul
                    rb = ps_small.tile([128, K], f32, name="rb", tag="rb")
                    nc.tensor.matmul(rb[:], ones1[:], rcp16[:])
                    rb16 = wpool.tile([128, K], f16, name="rb16")
                    nc.vector.tensor_copy(rb16[:], rb[:])

                    t4c = wpool.tile([128, 2, K], f16, name="t4c")
                    for c in range(2):
                        nc.vector.tensor_tensor(
                            t4c[:, c], tt[c][:], rb16[:], op=MULT)

                    # ---- s partials: sP = t4c (bcast over o) * ws, split
                    # per c so the c=0 eb4-matmul starts during the c=1 op ----
                    sP = wpool.tile([128, 2, O, K], f16, name="sP")
                    for c in range(2):
                        nc.vector.tensor_tensor(
                            sP[:, c],
                            t4c[:, c].unsqueeze(1).broadcast_to([128, O, K]),
                            ws_sb[:, c], op=MULT)

                # ---- c-sum + a2-group sum + replicate on PE:
                #      s4[(a2',b), k, o] = sum_c sum_a2 sP ----
                s4 = ps_s4.tile([128, K, O], f32, name="s4", tag="s4")
                for c in range(2):
                    nc.tensor.matmul(
                        s4[:], eb_sb[:], sP[:, c].transpose([0, 2, 1]),
                        start=(c == 0), stop=(c == 1))

                # ---- squash over k (on all 128 partitions) ----
                sq = wpool.tile([128, K, O], f16, name="sq")
                nc.scalar.square(sq[:], s4[:])
                ms = wpool.tile([128, O], f16, name="ms")
                nc.vector.tensor_reduce(ms[:], sq[:].transpose([0, 2, 1]),
                                        axis=AXX, op=ADD)
                mag = wpool.tile([128, O], f16, name="mag")
                nc.scalar.sqrt(mag[:], ms[:])
                den = wpool.tile([128, O], f16, name="den")
                nc.vector.tensor_scalar_add(den[:], ms[:], 1.0)
                rd = wpool.tile([128, O], f16, name="rd")
                nc.vector.reciprocal(rd[:], den[:])
                fb = wpool.tile([128, O], f16, name="fb")
                nc.vector.tensor_tensor(fb[:], mag[:], rd[:], op=MULT)

                if it == 2:
                    v4 = wpool.tile([BLOC, K, O], f16, name="vout")
                    nc.vector.tensor_tensor(
                        v4[:], s4[0:BLOC],
                        fb[0:BLOC].unsqueeze(1).broadcast_to([BLOC, K, O]),
                        op=MULT)
                    nc.sync.dma_start(out_d[:], v4[:])
                    continue

                v4 = wpool.tile([128, K, O], f16, name="v4")
                nc.vector.tensor_tensor(
                    v4[:], s4[:],
                    fb[:].unsqueeze(1).broadcast_to([128, K, O]), op=MULT)

                # ---- Gp[(a2,b), c, k] = sum_o wg * v.  The o-reduce is
                # split per c so matmul2's c=0 chain can start while the
                # c=1 half still reduces. ----
                gP = wpool.tile([128, 2, K, O], f16, name="gP")
                Gp = wpool.tile([128, 2, K], f16, name="Gp")
                for c in range(2):
                    nc.vector.tensor_tensor(
                        gP[:, c], v4[:], wg_sb[:, c], op=MULT)
                    nc.vector.tensor_reduce(
                        Gp[:, c], gP[:, c], axis=AXX, op=ADD)

                # ---- matmul2: bu[m, k] = sum_q xf[q, m] * Gp[q, k] ----
                # (mt-major: one PSUM accumulation group open at a time)
                bu = ps_bu.tile([128, MC, K], f32, name="bu", tag="bu")
                for mt in range(MC):
                    for c in range(2):
                        nc.tensor.matmul(
                            bu[:, mt, :],
                            xf_sb[:, c, mt * 128:(mt + 1) * 128],
                            Gp[:, c, :],
                            start=(c == 0), stop=(c == 1))

                # ---- AllReduce of (bu + b_prev/8) over the 8 cores ----
                cc_in = dpool.tile([128, MC, K], f32, name="cc_in")
                cc_out = dpool.tile([128, MC, K], f32, name="cc_out",
                                    addr_space="Shared")
                ar_prev, ar_sb = ar_sb, wpool.tile([128, MC, K], f32,
                                                   name="ar_sb")
                ccs = wpool.tile([128, MC, K], f32, name="ccs")
                if first:
                    nc.vector.tensor_copy(ccs[:], bu[:])
                else:
                    nc.vector.scalar_tensor_tensor(
                        ccs[:], ar_prev[:], 1.0 / NCORES, bu[:],
                        op0=MULT, op1=ADD)
                nc.sync.dma_start(cc_in[:], ccs[:])
                if USE_COLLECTIVES:
                    nc.gpsimd.collective_compute(
                        "AllReduce", ADD, replica_groups=RG,
                        ins=[cc_in[:].opt()], outs=[cc_out[:].opt()])
                    nc.sync.dma_start(ar_sb[:], cc_out[:])
                else:
                    nc.sync.dma_start(ar_sb[:], cc_in[:])

    nc.compile()
    return nc


def _host_prep(x):
    """Build the 8 per-core input maps from the full x [B, A, M]."""
    x = np.ascontiguousarray(x, dtype=np.float32)
    xt = x.reshape(B, M, A)  # faithful to reference's reshape (NOT a transpose)
    in_maps = []
    for i in range(NCORES):
        xi = xt[i * BLOC:(i + 1) * BLOC]              # [32, 1024, 8]
        # xm[p, mc, a*32+b]
        xm = xi.transpose(1, 2, 0).reshape(MC, 128, A, BLOC)
        xm = np.ascontiguousarray(
            xm.transpose(1, 0, 2, 3)).reshape(128, MC, 256).astype(np.float16)
        # xf[a2*32+b, c, m] with a = c*4+a2
        xf = xi.transpose(2, 0, 1).reshape(2, 4, BLOC, M)
        xf = np.ascontiguousarray(
            xf.transpose(1, 2, 0, 3)).reshape(128, 2, M).astype(np.float16)
        in_maps.append({"xm": xm, "xf": xf})
    return in_maps


def _host_w(W):
    """ws[(a2,b), c, o, k] = W[k, o, c*4+a2];
    wg[(a2,b), c, k, o] = W[k, o, c*4+a2] / B;
    eb4[(a2,b), (a2',b')] = (b == b')."""
    W = np.ascontiguousarray(W, dtype=np.float32)
    wss = W.reshape(K, O, 2, 4).transpose(3, 2, 1, 0)     # [a2, c, o, k]
    ws = np.ascontiguousarray(
        np.broadcast_to(wss[:, None], (4, BLOC, 2, O, K))).reshape(
            128, 2, O, K).astype(np.float16)
    wgs = (W / B).reshape(K, O, 2, 4).transpose(3, 2, 0, 1)  # [a2, c, k, o]
    wg = np.ascontiguousarray(
        np.broadcast_to(wgs[:, None], (4, BLOC, 2, K, O))).reshape(
            128, 2, K, O).astype(np.float16)
    eb4 = np.tile(np.eye(BLOC, dtype=np.float16), (4, 4))
    return {"ws": ws, "wg": wg, "eb4": eb4}


def _run(x, W, trace=False):
    from concourse import bass_utils

    if "nc" not in _prog_cache:
        _prog_cache["nc"] = _build_program()
    nc = _prog_cache["nc"]

    consts = _host_w(W)
    in_maps = _host_prep(x)
    for m in in_maps:
        m.update(consts)

    res = bass_utils.run_bass_kernel_spmd(
        nc, in_maps, core_ids=list(range(NCORES)), trace=trace)
    out = np.concatenate(
        [np.asarray(r["out"], np.float32) for r in res.results], axis=0)
    return out.reshape(B, K, O, 1), res


def kernel(x, W):
    out, _ = _run(x, W)
    return out


# revision 39
# speedup vs baseline: 1.3532x; 1.0439x over previous
"""Capsule-routing kernel for Trainium2 (8 NeuronCores, Bass/Tile).

Problem: nn_ClfCapsule — B=256, INPUT_ATOMS=8, MUL=1024, K=20, O=16, 3 routing
iterations.

u_hat[b,m,k,o] = sum_a W[k,o,a]*xt[b,m,a] (335 MB) is never materialized:
  t[b,k,a]  = sum_m c[m,k] * xt[b,m,a]          (TensorE, contract m=1024)
  s[b,k,o]  = sum_a W[k,o,a] * t[b,k,a]         (DVE mul + add, a2-sum on PE)
  v         = squash_over_k(s)
  g[b,k,a]  = sum_o W[k,o,a] * v[b,k,o]         (DVE mul + reduce)
  bU[m,k]   = sum_{b,a} xt[b,m,a] * g[b,k,a]    (TensorE, contract (b,a))
Data-parallel over batch (32/core); bU partials AllReduced after iters 1,2.

Perf design (fp32 baseline 89.2us -> 35.9us compute + 2 AllReduces):
- fp16 datapath: matmuls run 1 cycle/row instead of 4 (fp32), and DVE
  elementwise ops get the 2x/4x 16-bit modes.  fp16 (10-bit mantissa) keeps
  the pipeline rel-err ~1e-3, far under the 2e-2 gate (verified in numpy).
- softmax exp via a tuned degree-3 recurrence (4 DVE ops; |b| <= 0.9).
- softmax denominator via 8 accumulating PE matmuls (no DVE cross-chunk
  reduce); 1/denom broadcast by a ones-matmul.
- squash runs on all 128 partitions: the a2-group sum uses a [128,128]
  block-identity matmul that REPLICATES s into all 4 partition groups,
  removing the v-replication copies and the 32-partition squash.
- ACT engine is used only for sqrt; a dummy sqrt at t=0 preloads the
  activation table off the critical path.
- b_ij never lives on its own: the AllReduce input is bu + b_prev/8, so the
  post-collective path is DMA -> poly directly.
"""

import numpy as np

B, A, M = 256, 8, 1024
K, O = 20, 16
NCORES = 8
BLOC = B // NCORES  # 32
MC = M // 128       # 8 m-chunks

_prog_cache = {}
USE_COLLECTIVES = True  # debug switch: False replaces AllReduce with local copy

# e^x ~= (x*C1 + C0)*x + 1, least-squares fit of (e^x-1)/x on [-0.9, 0.9]
# (|b_ij| <= 0.84 empirically); pipeline rel err 6.6e-3 in the fp16 numpy
# sim vs the fp64 oracle (gate is 2e-2).
C1, C0 = 0.5206554848971008, 1.046132318580232


def _build_program(n_reps=1):
    import concourse.bacc as bacc
    import concourse.mybir as mybir
    import concourse.tile as tile

    f32 = mybir.dt.float32
    f16 = mybir.dt.float16
    nc = bacc.Bacc("TRN2", target_bir_lowering=False, debug=False,
                   num_devices=NCORES)

    # Host-prepped per-core DRAM inputs (see _host_prep / _host_w):
    #   xm[p, mc, q]      = xt[b, mc*128+p, a]   with q = a*32 + b
    #   xf[a2*32+b, c, m] = xt[b, m, c*4+a2]
    #   ws[(a2,b), c, o, k] = W[k, o, c*4+a2]
    #   wg[(a2,b), c, k, o] = W[k, o, c*4+a2] / 256
    #   eb4[(a2,b), (a2',b')] = (b == b')  (sums a2 groups AND replicates)
    xm_d = nc.dram_tensor("xm", [128, MC, 2 * 128], f16, kind="ExternalInput")
    xf_d = nc.dram_tensor("xf", [128, 2, M], f16, kind="ExternalInput")
    ws_d = nc.dram_tensor("ws", [128, 2, O, K], f16, kind="ExternalInput")
    wg_d = nc.dram_tensor("wg", [128, 2, K, O], f16, kind="ExternalInput")
    eb_d = nc.dram_tensor("eb4", [128, 128], f16, kind="ExternalInput")
    out_d = nc.dram_tensor("out", [BLOC, K, O], f16, kind="ExternalOutput")

    ADD = mybir.AluOpType.add
    MULT = mybir.AluOpType.mult
    AXX = mybir.AxisListType.X
    RG = [list(range(NCORES))]

    with tile.TileContext(nc) as tc:
        with (
            nc.allow_low_precision(
                reason="fp16 datapath validated vs fp64 oracle (rel ~1e-3)"),
            tc.tile_pool(name="const", bufs=1) as cpool,
            tc.tile_pool(name="work", bufs=2) as wpool,
            tc.tile_pool(name="ps_tt", bufs=2, space="PSUM") as ps_tt,
            tc.tile_pool(name="ps_small", bufs=1, space="PSUM") as ps_small,
            tc.tile_pool(name="ps_s4", bufs=2, space="PSUM") as ps_s4,
            tc.tile_pool(name="ps_bu", bufs=1, space="PSUM") as ps_bu,
            tc.tile_pool(name="dram", bufs=2, space="DRAM") as dpool,
        ):
            ones128 = cpool.tile([128, 1], f16)
            ones128f = cpool.tile([128, 1], f32)
            onesw = cpool.tile([128, 128], f16)
            eb_sb = cpool.tile([128, 128], f16)
            sqd = cpool.tile([1, 1], f32)
            nc.vector.memset(ones128[:], 1.0)
            nc.vector.memset(ones128f[:], 1.0)
            nc.vector.memset(onesw[:], 1.0)
            nc.vector.memset(sqd[:], 1.0)
            # dummy sqrt: pulls the ACT Sqrt table load off the critical path
            nc.scalar.sqrt(sqd[:], sqd[:])

            for _rep in range(n_reps):
              xm_sb = wpool.tile([128, MC, 256], f16, name="xm_sb")
              xf_sb = wpool.tile([128, 2, M], f16, name="xf_sb")
              ws_sb = wpool.tile([128, 2, O, K], f16, name="ws_sb")
              wg_sb = wpool.tile([128, 2, K, O], f16, name="wg_sb")

              # DMA-engine FIFO order target: xm, eb4, ws, xf, wg (by first
              # use).  xm/ws/xf issue on the SP/HWDGE path in that order;
              # eb4/wg ride the gpsimd SWDGE path, whose ~1us descgens slot
              # them into the FIFO right after xm and after xf respectively.
              nc.sync.dma_start(xm_sb[:], xm_d[:])
              nc.sync.dma_start(ws_sb[:], ws_d[:])
              nc.sync.dma_start(xf_sb[:], xf_d[:])
              nc.gpsimd.dma_start(eb_sb[:], eb_d[:])
              nc.gpsimd.dma_start(wg_sb[:], wg_d[:])

              ar_sb = None
              for it in range(3):
                first = (it == 0)

                if first:
                    # c uniform = 1/M: t1[q] = sum_m xm[m, q] / M, k-indep.
                    ttl = []
                    for c in range(2):
                        t_ = ps_tt.tile([128, 1], f32, name=f"ttl{c}", tag="tt")
                        for mc in range(MC):
                            nc.tensor.matmul(
                                t_[:], xm_sb[:, mc, c * 128:(c + 1) * 128],
                                ones128[:],
                                start=(mc == 0), stop=(mc == MC - 1))
                        ttl.append(t_)
                    t1sc = wpool.tile([128, 2], f32, name="t1sc")
                    for c in range(2):
                        nc.vector.tensor_scalar_mul(
                            t1sc[:, c:c + 1], ttl[c][:], 1.0 / M)
                    # sP via per-partition-scalar mult (TSP 4x mode)
                    sP = wpool.tile([128, 2, O, K], f16, name="sP")
                    for c in range(2):
                        nc.vector.tensor_scalar(
                            sP[:, c], ws_sb[:, c], t1sc[:, c:c + 1],
                            None, op0=MULT)

                else:
                    # ---- softmax numerator: e = (b*C1 + C0)*b + 1 with only
                    # TSP/TT ops (scalar_tensor_tensor never gets the 16-bit
                    # fast modes).
                    b16 = wpool.tile([128, MC, K], f16, name="b16")
                    nc.vector.tensor_scalar_mul(b16[:], ar_sb[:], 1.0)
                    q_sb = wpool.tile([128, MC, K], f16, name="q_sb")
                    nc.vector.tensor_scalar(
                        q_sb[:], b16[:], C1, C0, op0=MULT, op1=ADD)
                    bq = wpool.tile([128, MC, K], f16, name="bq")
                    nc.vector.tensor_tensor(bq[:], b16[:], q_sb[:], op=MULT)
                    e_sb = wpool.tile([128, MC, K], f16, name="e_sb")
                    nc.vector.tensor_scalar_add(e_sb[:], bq[:], 1.0)

                    # ---- denom, broadcast to all partitions in the same
                    # matmuls: dnb[p, k] = sum_m e (all-ones lhsT) ----
                    dnb = ps_small.tile([128, K], f32, name="dnb", tag="dn")
                    for mc in range(MC):
                        nc.tensor.matmul(dnb[:], onesw[:], e_sb[:, mc, :],
                                         start=(mc == 0), stop=(mc == MC - 1))
                    rb16 = wpool.tile([128, K], f16, name="rb16")
                    nc.vector.reciprocal(rb16[:], dnb[:])

                    # ---- matmul1 on unnormalized e
                    tt = []
                    for c in range(2):
                        t_ = ps_tt.tile([128, K], f32, name=f"tt{c}", tag="tt")
                        for mc in range(MC):
                            nc.tensor.matmul(
                                t_[:], xm_sb[:, mc, c * 128:(c + 1) * 128],
                                e_sb[:, mc, :],
                                start=(mc == 0), stop=(mc == MC - 1))
                        tt.append(t_)

                    # ---- t4c / sP interleaved per c so the c=0 eb4-matmul
                    # starts while the c=1 ops still run ----
                    t4c = wpool.tile([128, 2, K], f16, name="t4c")
                    sP = wpool.tile([128, 2, O, K], f16, name="sP")
                    for c in range(2):
                        nc.vector.tensor_tensor(
                            t4c[:, c], tt[c][:], rb16[:], op=MULT)
                        nc.vector.tensor_tensor(
                            sP[:, c],
                            t4c[:, c].unsqueeze(1).broadcast_to([128, O, K]),
                            ws_sb[:, c], op=MULT)

                # ---- c-sum + a2-group sum + replicate on PE:
                #      s4[(a2',b), k, o] = sum_c sum_a2 sP ----
                s4 = ps_s4.tile([128, K, O], f32, name="s4", tag="s4")
                for c in range(2):
                    nc.tensor.matmul(
                        s4[:], eb_sb[:], sP[:, c].transpose([0, 2, 1]),
                        start=(c == 0), stop=(c == 1))

                # ---- squash over k (on all 128 partitions).  One PSUM read
                # (s16 copy); everything after runs in the fp16 2x mode. ----
                s16 = wpool.tile([128, K, O], f16, name="s16")
                nc.vector.tensor_copy(s16[:], s4[:])
                sq = wpool.tile([128, K, O], f16, name="sq")
                nc.vector.tensor_tensor(sq[:], s16[:], s16[:], op=MULT)
                ms = wpool.tile([128, O], f16, name="ms")
                nc.vector.tensor_reduce(ms[:], sq[:].transpose([0, 2, 1]),
                                        axis=AXX, op=ADD)
                mag = wpool.tile([128, O], f16, name="mag")
                nc.scalar.sqrt(mag[:], ms[:])
                den = wpool.tile([128, O], f16, name="den")
                nc.vector.tensor_scalar_add(den[:], ms[:], 1.0)
                rd = wpool.tile([128, O], f16, name="rd")
                nc.vector.reciprocal(rd[:], den[:])
                fb = wpool.tile([128, O], f16, name="fb")
                nc.vector.tensor_tensor(fb[:], mag[:], rd[:], op=MULT)

                if it == 2:
                    v4 = wpool.tile([BLOC, K, O], f16, name="vout")
                    nc.vector.tensor_tensor(
                        v4[:], s16[0:BLOC],
                        fb[0:BLOC].unsqueeze(1).broadcast_to([BLOC, K, O]),
                        op=MULT)
                    nc.sync.dma_start(out_d[:], v4[:])
                    continue

                v4 = wpool.tile([128, K, O], f16, name="v4")
                nc.vector.tensor_tensor(
                    v4[:], s16[:],
                    fb[:].unsqueeze(1).broadcast_to([128, K, O]), op=MULT)

                # ---- Gp[(a2,b), c, k] = sum_o wg * v.  The o-reduce is
                # split per c so matmul2's c=0 chain can start while the
                # c=1 half still reduces. ----
                gP = wpool.tile([128, 2, K, O], f16, name="gP")
                Gp = wpool.tile([128, 2, K], f16, name="Gp")
                for c in range(2):
                    nc.vector.tensor_tensor(
                        gP[:, c], v4[:], wg_sb[:, c], op=MULT)
                    nc.vector.tensor_reduce(
                        Gp[:, c], gP[:, c], axis=AXX, op=ADD)

                # ---- matmul2: bu[m, k] = sum_q xf[q, m] * Gp[q, k] ----
                # (mt-major: one PSUM accumulation group open at a time)
                bu = ps_bu.tile([128, MC, K], f32, name="bu", tag="bu")
                for mt in range(MC):
                    for c in range(2):
                        nc.tensor.matmul(
                            bu[:, mt, :],
                            xf_sb[:, c, mt * 128:(mt + 1) * 128],
                            Gp[:, c, :],
                            start=(c == 0), stop=(c == 1))

                # ---- AllReduce of (bu + b_prev/8) over the 8 cores ----
                cc_in = dpool.tile([128, MC, K], f32, name="cc_in")
                cc_out = dpool.tile([128, MC, K], f32, name="cc_out",
                                    addr_space="Shared")
                ar_prev, ar_sb = ar_sb, wpool.tile([128, MC, K], f32,
                                                   name="ar_sb")
                ccs = wpool.tile([128, MC, K], f32, name="ccs")
                if first:
                    nc.vector.tensor_copy(ccs[:], bu[:])
                else:
                    nc.vector.scalar_tensor_tensor(
                        ccs[:], ar_prev[:], 1.0 / NCORES, bu[:],
                        op0=MULT, op1=ADD)
                nc.sync.dma_start(cc_in[:], ccs[:])
                if USE_COLLECTIVES:
                    nc.gpsimd.collective_compute(
                        "AllReduce", ADD, replica_groups=RG,
                        ins=[cc_in[:].opt()], outs=[cc_out[:].opt()])
                    nc.sync.dma_start(ar_sb[:], cc_out[:])
                else:
                    nc.sync.dma_start(ar_sb[:], cc_in[:])

    nc.compile()
    return nc


def _host_prep(x):
    """Build the 8 per-core input maps from the full x [B, A, M]."""
    x = np.ascontiguousarray(x, dtype=np.float32)
    xt = x.reshape(B, M, A)  # faithful to reference's reshape (NOT a transpose)
    in_maps = []
    for i in range(NCORES):
        xi = xt[i * BLOC:(i + 1) * BLOC]              # [32, 1024, 8]
        # xm[p, mc, a*32+b]
        xm = xi.transpose(1, 2, 0).reshape(MC, 128, A, BLOC)
        xm = np.ascontiguousarray(
            xm.transpose(1, 0, 2, 3)).reshape(128, MC, 256).astype(np.float16)
        # xf[a2*32+b, c, m] with a = c*4+a2
        xf = xi.transpose(2, 0, 1).reshape(2, 4, BLOC, M)
        xf = np.ascontiguousarray(
            xf.transpose(1, 2, 0, 3)).reshape(128, 2, M).astype(np.float16)
        in_maps.append({"xm": xm, "xf": xf})
    return in_maps


def _host_w(W):
    """ws[(a2,b), c, o, k] = W[k, o, c*4+a2];
    wg[(a2,b), c, k, o] = W[k, o, c*4+a2] / B;
    eb4[(a2,b), (a2',b')] = (b == b')."""
    W = np.ascontiguousarray(W, dtype=np.float32)
    wss = W.reshape(K, O, 2, 4).transpose(3, 2, 1, 0)     # [a2, c, o, k]
    ws = np.ascontiguousarray(
        np.broadcast_to(wss[:, None], (4, BLOC, 2, O, K))).reshape(
            128, 2, O, K).astype(np.float16)
    wgs = (W / B).reshape(K, O, 2, 4).transpose(3, 2, 0, 1)  # [a2, c, k, o]
    wg = np.ascontiguousarray(
        np.broadcast_to(wgs[:, None], (4, BLOC, 2, K, O))).reshape(
            128, 2, K, O).astype(np.float16)
    eb4 = np.tile(np.eye(BLOC, dtype=np.float16), (4, 4))
    return {"ws": ws, "wg": wg, "eb4": eb4}


def _run(x, W, trace=False):
    from concourse import bass_utils

    if "nc" not in _prog_cache:
        _prog_cache["nc"] = _build_program()
    nc = _prog_cache["nc"]

    consts = _host_w(W)
    in_maps = _host_prep(x)
    for m in in_maps:
        m.update(consts)

    res = bass_utils.run_bass_kernel_spmd(
        nc, in_maps, core_ids=list(range(NCORES)), trace=trace)
    out = np.concatenate(
        [np.asarray(r["out"], np.float32) for r in res.results], axis=0)
    return out.reshape(B, K, O, 1), res


def kernel(x, W):
    out, _ = _run(x, W)
    return out


# revision 44
# speedup vs baseline: 1.3575x; 1.0032x over previous
"""Capsule-routing kernel for Trainium2 (8 NeuronCores, Bass/Tile).

Problem: nn_ClfCapsule — B=256, INPUT_ATOMS=8, MUL=1024, K=20, O=16, 3 routing
iterations.

u_hat[b,m,k,o] = sum_a W[k,o,a]*xt[b,m,a] (335 MB) is never materialized:
  t[b,k,a]  = sum_m c[m,k] * xt[b,m,a]          (TensorE, contract m=1024)
  s[b,k,o]  = sum_a W[k,o,a] * t[b,k,a]         (DVE mul + add, a2-sum on PE)
  v         = squash_over_k(s)
  g[b,k,a]  = sum_o W[k,o,a] * v[b,k,o]         (DVE mul + reduce)
  bU[m,k]   = sum_{b,a} xt[b,m,a] * g[b,k,a]    (TensorE, contract (b,a))
Data-parallel over batch (32/core); bU partials AllReduced after iters 1,2.

Perf design (fp32 baseline 89.2us -> 35.9us compute + 2 AllReduces):
- fp16 datapath: matmuls run 1 cycle/row instead of 4 (fp32), and DVE
  elementwise ops get the 2x/4x 16-bit modes.  fp16 (10-bit mantissa) keeps
  the pipeline rel-err ~1e-3, far under the 2e-2 gate (verified in numpy).
- softmax exp via a tuned degree-3 recurrence (4 DVE ops; |b| <= 0.9).
- softmax denominator via 8 accumulating PE matmuls (no DVE cross-chunk
  reduce); 1/denom broadcast by a ones-matmul.
- squash runs on all 128 partitions: the a2-group sum uses a [128,128]
  block-identity matmul that REPLICATES s into all 4 partition groups,
  removing the v-replication copies and the 32-partition squash.
- ACT engine is used only for sqrt; a dummy sqrt at t=0 preloads the
  activation table off the critical path.
- b_ij never lives on its own: the AllReduce input is bu + b_prev/8, so the
  post-collective path is DMA -> poly directly.
"""

import numpy as np

B, A, M = 256, 8, 1024
K, O = 20, 16
NCORES = 8
BLOC = B // NCORES  # 32
MC = M // 128       # 8 m-chunks

_prog_cache = {}
USE_COLLECTIVES = True  # debug switch: False replaces AllReduce with local copy

# e^x ~= (x*C1 + C0)*x + 1, least-squares fit of (e^x-1)/x on [-0.9, 0.9]
# (|b_ij| <= 0.84 empirically); pipeline rel err 6.6e-3 in the fp16 numpy
# sim vs the fp64 oracle (gate is 2e-2).
C1, C0 = 0.5206554848971008, 1.046132318580232


def _build_program(n_reps=1):
    import concourse.bacc as bacc
    import concourse.mybir as mybir
    import concourse.tile as tile

    f32 = mybir.dt.float32
    f16 = mybir.dt.float16
    nc = bacc.Bacc("TRN2", target_bir_lowering=False, debug=False,
                   num_devices=NCORES)

    # Host-prepped per-core DRAM inputs (see _host_prep / _host_w):
    #   xm[p, mc, q]      = xt[b, mc*128+p, a]   with q = a*32 + b
    #   xf[a2*32+b, c, m] = xt[b, m, c*4+a2]
    #   ws[(a2,b), c, o, k] = W[k, o, c*4+a2]
    #   wg[(a2,b), c, k, o] = W[k, o, c*4+a2] / 256
    #   eb4[(a2,b), (a2',b')] = (b == b')  (sums a2 groups AND replicates)
    xm_d = nc.dram_tensor("xm", [128, MC, 2 * 128], f16, kind="ExternalInput")
    xf_d = nc.dram_tensor("xf", [128, 2, M], f16, kind="ExternalInput")
    ws_d = nc.dram_tensor("ws", [128, 2, O, K], f16, kind="ExternalInput")
    wg_d = nc.dram_tensor("wg", [128, 2, K, O], f16, kind="ExternalInput")
    eb_d = nc.dram_tensor("eb4", [128, 128], f16, kind="ExternalInput")
    out_d = nc.dram_tensor("out", [BLOC, K, O], f16, kind="ExternalOutput")

    SQRT = mybir.ActivationFunctionType.Sqrt
    ADD = mybir.AluOpType.add
    MULT = mybir.AluOpType.mult
    AXX = mybir.AxisListType.X
    RG = [list(range(NCORES))]

    with tile.TileContext(nc) as tc:
        with (
            nc.allow_low_precision(
                reason="fp16 datapath validated vs fp64 oracle (rel ~1e-3)"),
            tc.tile_pool(name="const", bufs=1) as cpool,
            tc.tile_pool(name="work", bufs=2) as wpool,
            tc.tile_pool(name="ps_tt", bufs=2, space="PSUM") as ps_tt,
            tc.tile_pool(name="ps_small", bufs=1, space="PSUM") as ps_small,
            tc.tile_pool(name="ps_s4", bufs=2, space="PSUM") as ps_s4,
            tc.tile_pool(name="ps_bu", bufs=1, space="PSUM") as ps_bu,
            tc.tile_pool(name="dram", bufs=2, space="DRAM") as dpool,
        ):
            ones128 = cpool.tile([128, 1], f16)
            ones128f = cpool.tile([128, 1], f32)
            negone = cpool.tile([128, 1], f32)
            nc.vector.memset(negone[:], -1.0)
            onesw = cpool.tile([128, 128], f16)
            eb_sb = cpool.tile([128, 128], f16)
            sqd = cpool.tile([1, 1], f32)
            nc.vector.memset(ones128[:], 1.0)
            nc.vector.memset(ones128f[:], 1.0)
            nc.vector.memset(onesw[:], 1.0)
            nc.vector.memset(sqd[:], 1.0)
            # dummy sqrt: pulls the ACT Sqrt table load off the critical path
            nc.scalar.sqrt(sqd[:], sqd[:])

            for _rep in range(n_reps):
              xm_sb = wpool.tile([128, MC, 256], f16, name="xm_sb")
              xf_sb = wpool.tile([128, 2, M], f16, name="xf_sb")
              ws_sb = wpool.tile([128, 2, O, K], f16, name="ws_sb")
              wg_sb = wpool.tile([128, 2, K, O], f16, name="wg_sb")

              # DMA-engine FIFO order target: xm, eb4, ws, xf, wg (by first
              # use).  xm/ws/xf issue on the SP/HWDGE path in that order;
              # eb4/wg ride the gpsimd SWDGE path, whose ~1us descgens slot
              # them into the FIFO right after xm and after xf respectively.
              nc.sync.dma_start(xm_sb[:], xm_d[:])
              nc.sync.dma_start(ws_sb[:], ws_d[:])
              nc.sync.dma_start(xf_sb[:], xf_d[:])
              nc.gpsimd.dma_start(eb_sb[:], eb_d[:])
              nc.gpsimd.dma_start(wg_sb[:], wg_d[:])

              ar_sb = None
              for it in range(3):
                first = (it == 0)

                if first:
                    # c uniform = 1/M: t1[q] = sum_m xm[m, q] / M, k-indep.
                    ttl = []
                    for c in range(2):
                        t_ = ps_tt.tile([128, 1], f32, name=f"ttl{c}", tag="tt")
                        for mc in range(MC):
                            nc.tensor.matmul(
                                t_[:], xm_sb[:, mc, c * 128:(c + 1) * 128],
                                ones128[:],
                                start=(mc == 0), stop=(mc == MC - 1))
                        ttl.append(t_)
                    t1sc = wpool.tile([128, 2], f32, name="t1sc")
                    for c in range(2):
                        nc.vector.tensor_scalar_mul(
                            t1sc[:, c:c + 1], ttl[c][:], 1.0 / M)
                    # sP via per-partition-scalar mult (TSP 4x mode)
                    sP = wpool.tile([128, 2, O, K], f16, name="sP")
                    for c in range(2):
                        nc.vector.tensor_scalar(
                            sP[:, c], ws_sb[:, c], t1sc[:, c:c + 1],
                            None, op0=MULT)

                else:
                    # ---- softmax numerator: e = (b*C1 + C0)*b + 1 with only
                    # TSP/TT ops (scalar_tensor_tensor never gets the 16-bit
                    # fast modes); both fp32 reads of ar_sb skip the cast.
                    q_sb = wpool.tile([128, MC, K], f16, name="q_sb")
                    nc.vector.tensor_scalar(
                        q_sb[:], ar_sb[:], C1, C0, op0=MULT, op1=ADD)
                    bq = wpool.tile([128, MC, K], f16, name="bq")
                    nc.vector.tensor_tensor(bq[:], ar_sb[:], q_sb[:], op=MULT)
                    e_sb = wpool.tile([128, MC, K], f16, name="e_sb")
                    nc.vector.tensor_scalar_add(e_sb[:], bq[:], 1.0)

                    # ---- denom, broadcast to all partitions in the same
                    # matmuls: dnb[p, k] = sum_m e (all-ones lhsT) ----
                    dnb = ps_small.tile([128, K], f32, name="dnb", tag="dn")
                    for mc in range(MC):
                        nc.tensor.matmul(dnb[:], onesw[:], e_sb[:, mc, :],
                                         start=(mc == 0), stop=(mc == MC - 1))
                    rb16 = wpool.tile([128, K], f16, name="rb16")
                    nc.vector.reciprocal(rb16[:], dnb[:])

                    # ---- matmul1 on unnormalized e
                    tt = []
                    for c in range(2):
                        t_ = ps_tt.tile([128, K], f32, name=f"tt{c}", tag="tt")
                        for mc in range(MC):
                            nc.tensor.matmul(
                                t_[:], xm_sb[:, mc, c * 128:(c + 1) * 128],
                                e_sb[:, mc, :],
                                start=(mc == 0), stop=(mc == MC - 1))
                        tt.append(t_)

                    # ---- t4c / sP interleaved per c so the c=0 eb4-matmul
                    # starts while the c=1 ops still run ----
                    t4c = wpool.tile([128, 2, K], f16, name="t4c")
                    sP = wpool.tile([128, 2, O, K], f16, name="sP")
                    for c in range(2):
                        nc.vector.tensor_tensor(
                            t4c[:, c], tt[c][:], rb16[:], op=MULT)
                        nc.vector.tensor_tensor(
                            sP[:, c],
                            t4c[:, c].unsqueeze(1).broadcast_to([128, O, K]),
                            ws_sb[:, c], op=MULT)

                # ---- c-sum + a2-group sum + replicate on PE:
                #      s4[(a2',b), k, o] = sum_c sum_a2 sP ----
                s4 = ps_s4.tile([128, K, O], f32, name="s4", tag="s4")
                for c in range(2):
                    nc.tensor.matmul(
                        s4[:], eb_sb[:], sP[:, c].transpose([0, 2, 1]),
                        start=(c == 0), stop=(c == 1))

                # ---- squash over k (on all 128 partitions).  One PSUM read
                # (s16 copy); everything after runs in the fp16 2x mode.
                # sq carries a constant ones-row so the k-reduce yields
                # den = 1 + sum_k s^2 directly; mag = sqrt(den - 1) via the
                # ACT bias. ----
                s16 = wpool.tile([128, K, O], f16, name="s16")
                nc.vector.tensor_copy(s16[:], s4[:])
                sq = wpool.tile([128, K + 1, O], f16, name="sq")
                nc.vector.memset(sq[:, K, :], 1.0)
                nc.vector.tensor_tensor(sq[:, 0:K], s16[:], s16[:], op=MULT)
                den = wpool.tile([128, O], f16, name="den")
                nc.vector.tensor_reduce(den[:], sq[:].transpose([0, 2, 1]),
                                        axis=AXX, op=ADD)
                mag = wpool.tile([128, O], f16, name="mag")
                nc.scalar.activation(mag[:], den[:], SQRT, bias=negone[:])
                rd = wpool.tile([128, O], f16, name="rd")
                nc.vector.reciprocal(rd[:], den[:])
                fb = wpool.tile([128, O], f16, name="fb")
                nc.vector.tensor_tensor(fb[:], mag[:], rd[:], op=MULT)

                if it == 2:
                    v4 = wpool.tile([BLOC, K, O], f16, name="vout")
                    nc.vector.tensor_tensor(
                        v4[:], s16[0:BLOC],
                        fb[0:BLOC].unsqueeze(1).broadcast_to([BLOC, K, O]),
                        op=MULT)
                    nc.sync.dma_start(out_d[:], v4[:])
                    continue

                v4 = wpool.tile([128, K, O], f16, name="v4")
                nc.vector.tensor_tensor(
                    v4[:], s16[:],
                    fb[:].unsqueeze(1).broadcast_to([128, K, O]), op=MULT)

                # ---- Gp[(a2,b), c, k] = sum_o wg * v.  The o-reduce is
                # split per c so matmul2's c=0 chain can start while the
                # c=1 half still reduces. ----
                gP = wpool.tile([128, 2, K, O], f16, name="gP")
                Gp = wpool.tile([128, 2, K], f16, name="Gp")
                for c in range(2):
                    nc.vector.tensor_tensor(
                        gP[:, c], v4[:], wg_sb[:, c], op=MULT)
                    nc.vector.tensor_reduce(
                        Gp[:, c], gP[:, c], axis=AXX, op=ADD)

                # ---- matmul2: bu[m, k] = sum_q xf[q, m] * Gp[q, k] ----
                # (mt-major: one PSUM accumulation group open at a time)
                bu = ps_bu.tile([128, MC, K], f32, name="bu", tag="bu")
                for mt in range(MC):
                    for c in range(2):
                        nc.tensor.matmul(
                            bu[:, mt, :],
                            xf_sb[:, c, mt * 128:(mt + 1) * 128],
                            Gp[:, c, :],
                            start=(c == 0), stop=(c == 1))

                # ---- AllReduce of (bu + b_prev/8) over the 8 cores ----
                cc_in = dpool.tile([128, MC, K], f32, name="cc_in")
                cc_out = dpool.tile([128, MC, K], f32, name="cc_out",
                                    addr_space="Shared")
                ar_prev, ar_sb = ar_sb, wpool.tile([128, MC, K], f32,
                                                   name="ar_sb")
                ccs = wpool.tile([128, MC, K], f32, name="ccs")
                if first:
                    nc.vector.tensor_copy(ccs[:], bu[:])
                else:
                    nc.vector.scalar_tensor_tensor(
                        ccs[:], ar_prev[:], 1.0 / NCORES, bu[:],
                        op0=MULT, op1=ADD)
                nc.sync.dma_start(cc_in[:], ccs[:])
                if USE_COLLECTIVES:
                    nc.gpsimd.collective_compute(
                        "AllReduce", ADD, replica_groups=RG,
                        ins=[cc_in[:].opt()], outs=[cc_out[:].opt()])
                    nc.sync.dma_start(ar_sb[:], cc_out[:])
                else:
                    nc.sync.dma_start(ar_sb[:], cc_in[:])

    nc.compile()
    return nc


def _host_prep(x):
    """Build the 8 per-core input maps from the full x [B, A, M]."""
    x = np.ascontiguousarray(x, dtype=np.float32)
    xt = x.reshape(B, M, A)  # faithful to reference's reshape (NOT a transpose)
    in_maps = []
    for i in range(NCORES):
        xi = xt[i * BLOC:(i + 1) * BLOC]              # [32, 1024, 8]
        # xm[p, mc, a*32+b]
        xm = xi.transpose(1, 2, 0).reshape(MC, 128, A, BLOC)
        xm = np.ascontiguousarray(
            xm.transpose(1, 0, 2, 3)).reshape(128, MC, 256).astype(np.float16)
        # xf[a2*32+b, c, m] with a = c*4+a2
        xf = xi.transpose(2, 0, 1).reshape(2, 4, BLOC, M)
        xf = np.ascontiguousarray(
            xf.transpose(1, 2, 0, 3)).reshape(128, 2, M).astype(np.float16)
        in_maps.append({"xm": xm, "xf": xf})
    return in_maps


def _host_w(W):
    """ws[(a2,b), c, o, k] = W[k, o, c*4+a2];
    wg[(a2,b), c, k, o] = W[k, o, c*4+a2] / B;
    eb4[(a2,b), (a2',b')] = (b == b')."""
    W = np.ascontiguousarray(W, dtype=np.float32)
    wss = W.reshape(K, O, 2, 4).transpose(3, 2, 1, 0)     # [a2, c, o, k]
    ws = np.ascontiguousarray(
        np.broadcast_to(wss[:, None], (4, BLOC, 2, O, K))).reshape(
            128, 2, O, K).astype(np.float16)
    wgs = (W / B).reshape(K, O, 2, 4).transpose(3, 2, 0, 1)  # [a2, c, k, o]
    wg = np.ascontiguousarray(
        np.broadcast_to(wgs[:, None], (4, BLOC, 2, K, O))).reshape(
            128, 2, K, O).astype(np.float16)
    eb4 = np.tile(np.eye(BLOC, dtype=np.float16), (4, 4))
    return {"ws": ws, "wg": wg, "eb4": eb4}


def _run(x, W, trace=False):
    from concourse import bass_utils

    if "nc" not in _prog_cache:
        _prog_cache["nc"] = _build_program()
    nc = _prog_cache["nc"]

    consts = _host_w(W)
    in_maps = _host_prep(x)
    for m in in_maps:
        m.update(consts)

    res = bass_utils.run_bass_kernel_spmd(
        nc, in_maps, core_ids=list(range(NCORES)), trace=trace)
    out = np.concatenate(
        [np.asarray(r["out"], np.float32) for r in res.results], axis=0)
    return out.reshape(B, K, O, 1), res


def kernel(x, W):
    out, _ = _run(x, W)
    return out


# revision 47
# speedup vs baseline: 1.3669x; 1.0069x over previous
"""Capsule-routing kernel for Trainium2 (8 NeuronCores, Bass/Tile).

Problem: nn_ClfCapsule — B=256, INPUT_ATOMS=8, MUL=1024, K=20, O=16, 3 routing
iterations.

u_hat[b,m,k,o] = sum_a W[k,o,a]*xt[b,m,a] (335 MB) is never materialized:
  t[b,k,a]  = sum_m c[m,k] * xt[b,m,a]          (TensorE, contract m=1024)
  s[b,k,o]  = sum_a W[k,o,a] * t[b,k,a]         (DVE mul + add, a2-sum on PE)
  v         = squash_over_k(s)
  g[b,k,a]  = sum_o W[k,o,a] * v[b,k,o]         (DVE mul + reduce)
  bU[m,k]   = sum_{b,a} xt[b,m,a] * g[b,k,a]    (TensorE, contract (b,a))
Data-parallel over batch (32/core); bU partials AllReduced after iters 1,2.

Perf design (fp32 baseline 89.2us -> 35.9us compute + 2 AllReduces):
- fp16 datapath: matmuls run 1 cycle/row instead of 4 (fp32), and DVE
  elementwise ops get the 2x/4x 16-bit modes.  fp16 (10-bit mantissa) keeps
  the pipeline rel-err ~1e-3, far under the 2e-2 gate (verified in numpy).
- softmax exp via a tuned degree-3 recurrence (4 DVE ops; |b| <= 0.9).
- softmax denominator via 8 accumulating PE matmuls (no DVE cross-chunk
  reduce); 1/denom broadcast by a ones-matmul.
- squash runs on all 128 partitions: the a2-group sum uses a [128,128]
  block-identity matmul that REPLICATES s into all 4 partition groups,
  removing the v-replication copies and the 32-partition squash.
- ACT engine is used only for sqrt; a dummy sqrt at t=0 preloads the
  activation table off the critical path.
- b_ij never lives on its own: the AllReduce input is bu + b_prev/8, so the
  post-collective path is DMA -> poly directly.
"""

import numpy as np

B, A, M = 256, 8, 1024
K, O = 20, 16
NCORES = 8
BLOC = B // NCORES  # 32
MC = M // 128       # 8 m-chunks

_prog_cache = {}
USE_COLLECTIVES = True  # debug switch: False replaces AllReduce with local copy

# e^x ~= (x*C1 + C0)*x + 1, least-squares fit of (e^x-1)/x on [-0.9, 0.9]
# (|b_ij| <= 0.84 empirically); pipeline rel err 6.6e-3 in the fp16 numpy
# sim vs the fp64 oracle (gate is 2e-2).
C1, C0 = 0.5206554848971008, 1.046132318580232


def _build_program(n_reps=1):
    import concourse.bacc as bacc
    import concourse.mybir as mybir
    import concourse.tile as tile

    f32 = mybir.dt.float32
    f16 = mybir.dt.float16
    nc = bacc.Bacc("TRN2", target_bir_lowering=False, debug=False,
                   num_devices=NCORES)

    # Host-prepped per-core DRAM inputs (see _host_prep / _host_w):
    #   xm[p, mc, q]      = xt[b, mc*128+p, a]   with q = a*32 + b
    #   xf[a2*32+b, c, m] = xt[b, m, c*4+a2]
    #   ws[(a2,b), c, o, k] = W[k, o, c*4+a2]
    #   wg[(a2,b), c, k, o] = W[k, o, c*4+a2] / 256
    #   eb4[(a2,b), (a2',b')] = (b == b')  (sums a2 groups AND replicates)
    xm_d = nc.dram_tensor("xm", [128, MC, 2 * 128], f16, kind="ExternalInput")
    xf_d = nc.dram_tensor("xf", [128, 2, M], f16, kind="ExternalInput")
    ws_d = nc.dram_tensor("ws", [128, 2, O, K], f16, kind="ExternalInput")
    wg_d = nc.dram_tensor("wg", [128, 2, K, O], f16, kind="ExternalInput")
    eb_d = nc.dram_tensor("eb4", [128, 128], f16, kind="ExternalInput")
    out_d = nc.dram_tensor("out", [BLOC, K, O], f16, kind="ExternalOutput")

    SQRT = mybir.ActivationFunctionType.Sqrt
    ADD = mybir.AluOpType.add
    MULT = mybir.AluOpType.mult
    AXX = mybir.AxisListType.X
    RG = [list(range(NCORES))]

    with tile.TileContext(nc) as tc:
        with (
            nc.allow_low_precision(
                reason="fp16 datapath validated vs fp64 oracle (rel ~1e-3)"),
            tc.tile_pool(name="const", bufs=1) as cpool,
            tc.tile_pool(name="work", bufs=2) as wpool,
            tc.tile_pool(name="ps_tt", bufs=2, space="PSUM") as ps_tt,
            tc.tile_pool(name="ps_small", bufs=1, space="PSUM") as ps_small,
            tc.tile_pool(name="ps_s4", bufs=2, space="PSUM") as ps_s4,
            tc.tile_pool(name="ps_bu", bufs=1, space="PSUM") as ps_bu,
            tc.tile_pool(name="dram", bufs=2, space="DRAM") as dpool,
        ):
            ones128 = cpool.tile([128, 1], f16)
            ones128f = cpool.tile([128, 1], f32)
            negone = cpool.tile([128, 1], f32)
            nc.vector.memset(negone[:], -1.0)
            onesw = cpool.tile([128, 128], f16)
            eb_sb = cpool.tile([128, 128], f16)
            sqd = cpool.tile([1, 1], f32)
            nc.vector.memset(ones128[:], 1.0)
            nc.vector.memset(ones128f[:], 1.0)
            nc.vector.memset(onesw[:], 1.0)
            nc.vector.memset(sqd[:], 1.0)
            # dummy sqrt: pulls the ACT Sqrt table load off the critical path
            nc.scalar.sqrt(sqd[:], sqd[:])

            for _rep in range(n_reps):
              xm_sb = wpool.tile([128, MC, 256], f16, name="xm_sb")
              xf_sb = wpool.tile([128, 2, M], f16, name="xf_sb")
              ws_sb = wpool.tile([128, 2, O, K], f16, name="ws_sb")
              wg_sb = wpool.tile([128, 2, K, O], f16, name="wg_sb")

              # DMA-engine FIFO order target: xm, eb4, ws, xf, wg (by first
              # use).  xm/ws/xf issue on the SP/HWDGE path in that order;
              # eb4/wg ride the gpsimd SWDGE path, whose ~1us descgens slot
              # them into the FIFO right after xm and after xf respectively.
              nc.sync.dma_start(xm_sb[:], xm_d[:])
              nc.sync.dma_start(ws_sb[:], ws_d[:])
              nc.sync.dma_start(xf_sb[:], xf_d[:])
              nc.gpsimd.dma_start(eb_sb[:], eb_d[:])
              nc.gpsimd.dma_start(wg_sb[:], wg_d[:])

              ar_sb = None
              for it in range(3):
                first = (it == 0)

                if first:
                    # c uniform = 1/M: t1[q] = sum_m xm[m, q] / M, k-indep.
                    ttl = []
                    for c in range(2):
                        t_ = ps_tt.tile([128, 1], f32, name=f"ttl{c}", tag="tt")
                        for mc in range(MC):
                            nc.tensor.matmul(
                                t_[:], xm_sb[:, mc, c * 128:(c + 1) * 128],
                                ones128[:],
                                start=(mc == 0), stop=(mc == MC - 1))
                        ttl.append(t_)
                    # keep PE busy through the t1sc/sP window so the s4 pair
                    # doesn't drop to the cold p-state tier (results unused)
                    warm = ps_small.tile([1, K], f32, name="warm", tag="warm")
                    for w in range(40):
                        nc.tensor.matmul(
                            warm[:], ones128[:], xm_sb[:, w % MC, 0:K],
                            start=(w == 0), stop=(w == 39))
                    t1sc = wpool.tile([128, 2], f32, name="t1sc")
                    for c in range(2):
                        nc.vector.tensor_scalar_mul(
                            t1sc[:, c:c + 1], ttl[c][:], 1.0 / M)
                    # sP via per-partition-scalar mult (TSP 4x mode)
                    sP = wpool.tile([128, 2, O, K], f16, name="sP")
                    for c in range(2):
                        nc.vector.tensor_scalar(
                            sP[:, c], ws_sb[:, c], t1sc[:, c:c + 1],
                            None, op0=MULT)

                else:
                    # ---- softmax numerator: e = (b*C1 + C0)*b + 1 with only
                    # TSP/TT ops (scalar_tensor_tensor never gets the 16-bit
                    # fast modes); both fp32 reads of ar_sb skip the cast.
                    q_sb = wpool.tile([128, MC, K], f16, name="q_sb")
                    nc.vector.tensor_scalar(
                        q_sb[:], ar_sb[:], C1, C0, op0=MULT, op1=ADD)
                    bq = wpool.tile([128, MC, K], f16, name="bq")
                    nc.vector.tensor_tensor(bq[:], ar_sb[:], q_sb[:], op=MULT)
                    e_sb = wpool.tile([128, MC, K], f16, name="e_sb")
                    nc.vector.tensor_scalar_add(e_sb[:], bq[:], 1.0)

                    # ---- denom, broadcast to all partitions in the same
                    # matmuls: dnb[p, k] = sum_m e (all-ones lhsT) ----
                    dnb = ps_small.tile([128, K], f32, name="dnb", tag="dn")
                    for mc in range(MC):
                        nc.tensor.matmul(dnb[:], onesw[:], e_sb[:, mc, :],
                                         start=(mc == 0), stop=(mc == MC - 1))
                    rb16 = wpool.tile([128, K], f16, name="rb16")
                    nc.vector.reciprocal(rb16[:], dnb[:])

                    # ---- matmul1 on unnormalized e
                    tt = []
                    for c in range(2):
                        t_ = ps_tt.tile([128, K], f32, name=f"tt{c}", tag="tt")
                        for mc in range(MC):
                            nc.tensor.matmul(
                                t_[:], xm_sb[:, mc, c * 128:(c + 1) * 128],
                                e_sb[:, mc, :],
                                start=(mc == 0), stop=(mc == MC - 1))
                        tt.append(t_)

                    # ---- t4c / sP interleaved per c so the c=0 eb4-matmul
                    # starts while the c=1 ops still run ----
                    t4c = wpool.tile([128, 2, K], f16, name="t4c")
                    sP = wpool.tile([128, 2, O, K], f16, name="sP")
                    for c in range(2):
                        nc.vector.tensor_tensor(
                            t4c[:, c], tt[c][:], rb16[:], op=MULT)
                        nc.vector.tensor_tensor(
                            sP[:, c],
                            t4c[:, c].unsqueeze(1).broadcast_to([128, O, K]),
                            ws_sb[:, c], op=MULT)

                # ---- c-sum + a2-group sum + replicate on PE:
                #      s4[(a2',b), k, o] = sum_c sum_a2 sP ----
                s4 = ps_s4.tile([128, K, O], f32, name="s4", tag="s4")
                for c in range(2):
                    nc.tensor.matmul(
                        s4[:], eb_sb[:], sP[:, c].transpose([0, 2, 1]),
                        start=(c == 0), stop=(c == 1))

                # ---- squash over k (on all 128 partitions).  One PSUM read
                # (s16 copy); everything after runs in the fp16 2x mode.
                # sq carries a constant ones-row so the k-reduce yields
                # den = 1 + sum_k s^2 directly; mag = sqrt(den - 1) via the
                # ACT bias. ----
                s16 = wpool.tile([128, K, O], f16, name="s16")
                nc.vector.tensor_copy(s16[:], s4[:])
                sq = wpool.tile([128, K + 1, O], f16, name="sq")
                nc.vector.memset(sq[:, K, :], 1.0)
                nc.vector.tensor_tensor(sq[:, 0:K], s16[:], s16[:], op=MULT)
                den = wpool.tile([128, O], f16, name="den")
                nc.vector.tensor_reduce(den[:], sq[:].transpose([0, 2, 1]),
                                        axis=AXX, op=ADD)
                mag = wpool.tile([128, O], f16, name="mag")
                nc.scalar.activation(mag[:], den[:], SQRT, bias=negone[:])
                rd = wpool.tile([128, O], f16, name="rd")
                nc.vector.reciprocal(rd[:], den[:])
                fb = wpool.tile([128, O], f16, name="fb")
                nc.vector.tensor_tensor(fb[:], mag[:], rd[:], op=MULT)

                if it == 2:
                    v4 = wpool.tile([BLOC, K, O], f16, name="vout")
                    nc.vector.tensor_tensor(
                        v4[:], s16[0:BLOC],
                        fb[0:BLOC].unsqueeze(1).broadcast_to([BLOC, K, O]),
                        op=MULT)
                    nc.sync.dma_start(out_d[:], v4[:])
                    continue

                v4 = wpool.tile([128, K, O], f16, name="v4")
                nc.vector.tensor_tensor(
                    v4[:], s16[:],
                    fb[:].unsqueeze(1).broadcast_to([128, K, O]), op=MULT)

                # ---- Gp[(a2,b), c, k] = sum_o wg * v.  The o-reduce is
                # split per c so matmul2's c=0 chain can start while the
                # c=1 half still reduces. ----
                gP = wpool.tile([128, 2, K, O], f16, name="gP")
                Gp = wpool.tile([128, 2, K], f16, name="Gp")
                for c in range(2):
                    nc.vector.tensor_tensor(
                        gP[:, c], v4[:], wg_sb[:, c], op=MULT)
                    nc.vector.tensor_reduce(
                        Gp[:, c], gP[:, c], axis=AXX, op=ADD)

                # ---- matmul2: bu[m, k] = sum_q xf[q, m] * Gp[q, k] ----
                # (mt-major: one PSUM accumulation group open at a time)
                bu = ps_bu.tile([128, MC, K], f32, name="bu", tag="bu")
                for mt in range(MC):
                    for c in range(2):
                        nc.tensor.matmul(
                            bu[:, mt, :],
                            xf_sb[:, c, mt * 128:(mt + 1) * 128],
                            Gp[:, c, :],
                            start=(c == 0), stop=(c == 1))

                # ---- AllReduce of (bu + b_prev/8) over the 8 cores ----
                cc_in = dpool.tile([128, MC, K], f32, name="cc_in")
                cc_out = dpool.tile([128, MC, K], f32, name="cc_out",
                                    addr_space="Shared")
                ar_prev, ar_sb = ar_sb, wpool.tile([128, MC, K], f32,
                                                   name="ar_sb")
                ccs = wpool.tile([128, MC, K], f32, name="ccs")
                if first:
                    nc.vector.tensor_copy(ccs[:], bu[:])
                else:
                    nc.vector.scalar_tensor_tensor(
                        ccs[:], ar_prev[:], 1.0 / NCORES, bu[:],
                        op0=MULT, op1=ADD)
                nc.sync.dma_start(cc_in[:], ccs[:])
                if USE_COLLECTIVES:
                    nc.gpsimd.collective_compute(
                        "AllReduce", ADD, replica_groups=RG,
                        ins=[cc_in[:].opt()], outs=[cc_out[:].opt()])
                    nc.sync.dma_start(ar_sb[:], cc_out[:])
                else:
                    nc.sync.dma_start(ar_sb[:], cc_in[:])

    nc.compile()
    return nc


def _host_prep(x):
    """Build the 8 per-core input maps from the full x [B, A, M]."""
    x = np.ascontiguousarray(x, dtype=np.float32)
    xt = x.reshape(B, M, A)  # faithful to reference's reshape (NOT a transpose)
    in_maps = []
    for i in range(NCORES):
        xi = xt[i * BLOC:(i + 1) * BLOC]              # [32, 1024, 8]
        # xm[p, mc, a*32+b]
        xm = xi.transpose(1, 2, 0).reshape(MC, 128, A, BLOC)
        xm = np.ascontiguousarray(
            xm.transpose(1, 0, 2, 3)).reshape(128, MC, 256).astype(np.float16)
        # xf[a2*32+b, c, m] with a = c*4+a2
        xf = xi.transpose(2, 0, 1).reshape(2, 4, BLOC, M)
        xf = np.ascontiguousarray(
            xf.transpose(1, 2, 0, 3)).reshape(128, 2, M).astype(np.float16)
        in_maps.append({"xm": xm, "xf": xf})
    return in_maps


def _host_w(W):
    """ws[(a2,b), c, o, k] = W[k, o, c*4+a2];
    wg[(a2,b), c, k, o] = W[k, o, c*4+a2] / B;
    eb4[(a2,b), (a2',b')] = (b == b')."""
    W = np.ascontiguousarray(W, dtype=np.float32)
    wss = W.reshape(K, O, 2, 4).transpose(3, 2, 1, 0)     # [a2, c, o, k]
    ws = np.ascontiguousarray(
        np.broadcast_to(wss[:, None], (4, BLOC, 2, O, K))).reshape(
            128, 2, O, K).astype(np.float16)
    wgs = (W / B).reshape(K, O, 2, 4).transpose(3, 2, 0, 1)  # [a2, c, k, o]
    wg = np.ascontiguousarray(
        np.broadcast_to(wgs[:, None], (4, BLOC, 2, K, O))).reshape(
            128, 2, K, O).astype(np.float16)
    eb4 = np.tile(np.eye(BLOC, dtype=np.float16), (4, 4))
    return {"ws": ws, "wg": wg, "eb4": eb4}


def _run(x, W, trace=False):
    from concourse import bass_utils

    if "nc" not in _prog_cache:
        _prog_cache["nc"] = _build_program()
    nc = _prog_cache["nc"]

    consts = _host_w(W)
    in_maps = _host_prep(x)
    for m in in_maps:
        m.update(consts)

    res = bass_utils.run_bass_kernel_spmd(
        nc, in_maps, core_ids=list(range(NCORES)), trace=trace)
    out = np.concatenate(
        [np.asarray(r["out"], np.float32) for r in res.results], axis=0)
    return out.reshape(B, K, O, 1), res


def kernel(x, W):
    out, _ = _run(x, W)
    return out
